# revision 1
# baseline (speedup 1.0000x reference)
"""Grouped SwiGLU experts (MoE, contiguous per-expert token segments) on 8 trn2 cores.

Strategy: expert-parallel over 512-token work units ("slots").  Host splits the
token rows into per-expert contiguous segments (exactly mirroring the
reference's searchsorted routing), chops each segment into 512-token chunks,
and deals chunks round the 8 cores contiguously so every core runs the same
static program: S slots x [512 tokens x one expert].  Per slot the device
computes  out = (silu(x @ w1) * (x @ w3)) @ w2  with bf16 matmuls (fp32 PSUM
accumulation).  x is pre-transposed on host so no on-device transposes are
needed; h = silu(g1)*g3 is produced directly in [hidden, token] layout which
is exactly the lhsT layout the w2 matmul wants.

kernel(**inputs) -> full [16384, 2048] fp32 output.  Self-contained.
"""

import math
import os

import numpy as np
import ml_dtypes

import concourse.bass as bass
import concourse.tile as tile
from concourse import bacc
from concourse import mybir
from concourse.bass_utils import run_bass_kernel_spmd

N_CORES = 8
D = 2048          # dim_in
H = 1408          # dim_hidden
TOK = 512         # tokens per slot
P = 128           # partitions
D_T = D // P      # 16 d-chunks
H_T = H // P      # 11 hid-chunks
TK = TOK // P     # 4 token tiles per slot

_compiled_cache = {}
last_run_info = {}


def _build_program(S: int, cdt):
    """Per-core SPMD program: S slots, each 512 tokens of one expert."""
    nc = bacc.Bacc()

    xt = nc.declare_dram_parameter("xt", [D, S * TOK], cdt, isOutput=False)
    w1 = nc.declare_dram_parameter("w1", [S, D, H], cdt, isOutput=False)
    w3 = nc.declare_dram_parameter("w3", [S, D, H], cdt, isOutput=False)
    w2 = nc.declare_dram_parameter("w2", [S, H, D], cdt, isOutput=False)
    out = nc.declare_dram_parameter("out", [S * TOK, D], mybir.dt.float32, isOutput=True)

    # hidden-dim split of w1/w3: "lo" = hid chunks [0, H_LO), "hi" = rest.
    # lo tiles are last read at hg == H_LO-1, so next slot's lo prefetch can
    # start mid-phase-1 instead of at phase-1 end.
    H_LO = 7
    LOW = H_LO * P
    HIW = H - LOW

    with tile.TileContext(nc) as tc:
        with (
            tc.tile_pool(name="xtp", bufs=2) as xtp,
            tc.tile_pool(name="wp", bufs=1) as wp,
            tc.tile_pool(name="hp", bufs=2) as hp,
            tc.tile_pool(name="w2p", bufs=1) as w2p,
            tc.tile_pool(name="outp", bufs=3) as outp,
            tc.tile_pool(name="tmp", bufs=2) as tmp,
            tc.tile_pool(name="ps", bufs=8, space="PSUM") as psp,
        ):
            for s in range(S):
                # ---- loads for this slot (per-d-chunk tiles, lo then hi) ----
                xt_sb = [None] * D_T
                w1lo = [None] * D_T
                w3lo = [None] * D_T
                w1hi = [None] * D_T
                w3hi = [None] * D_T
                for d in range(D_T):
                    dp = slice(d * P, (d + 1) * P)
                    xt_sb[d] = xtp.tile([P, TOK], cdt, tag=f"xt{d}", bufs=2,
                                        name=f"xt_{s}_{d}")
                    nc.sync.dma_start(out=xt_sb[d][:],
                                      in_=xt[dp, s * TOK:(s + 1) * TOK])
                    w1lo[d] = wp.tile([P, LOW], cdt, tag=f"w1lo{d}",
                                      name=f"w1lo_{s}_{d}")
                    nc.sync.dma_start(out=w1lo[d][:], in_=w1[s, dp, 0:LOW])
                    w3lo[d] = wp.tile([P, LOW], cdt, tag=f"w3lo{d}",
                                      name=f"w3lo_{s}_{d}")
                    nc.sync.dma_start(out=w3lo[d][:], in_=w3[s, dp, 0:LOW])
                for d in range(D_T):
                    dp = slice(d * P, (d + 1) * P)
                    w1hi[d] = wp.tile([P, HIW], cdt, tag=f"w1hi{d}",
                                      name=f"w1hi_{s}_{d}")
                    nc.sync.dma_start(out=w1hi[d][:], in_=w1[s, dp, LOW:H])
                    w3hi[d] = wp.tile([P, HIW], cdt, tag=f"w3hi{d}",
                                      name=f"w3hi_{s}_{d}")
                    nc.sync.dma_start(out=w3hi[d][:], in_=w3[s, dp, LOW:H])
                # w2 loads (per hid-chunk tiles; dh0 half can fully prefetch
                # during phase 1, dh1 reuses the tag's slot after dh0 drains)
                w2_sb = [[None] * H_T for _ in range(2)]
                for dh in range(2):
                    dsl = slice(dh * (D // 2), (dh + 1) * (D // 2))
                    for hc in range(H_T):
                        w2_sb[dh][hc] = w2p.tile([P, D // 2], cdt, tag=f"w2_{hc}",
                                                 name=f"w2sb_{s}_{dh}_{hc}")
                        nc.sync.dma_start(
                            out=w2_sb[dh][hc][:], in_=w2[s, hc * P:(hc + 1) * P, dsl]
                        )

                # ---- phase 1: h[hid, tok] = silu(w1.T x) * (w3.T x) ----
                h_sb = hp.tile([P, H_T, TOK], cdt, tag="h")
                for hg in range(H_T):
                    if hg < H_LO:
                        wa, wb = w1lo, w3lo
                        hsl = slice(hg * P, (hg + 1) * P)
                    else:
                        wa, wb = w1hi, w3hi
                        hsl = slice((hg - H_LO) * P, (hg - H_LO + 1) * P)
                    ps1 = psp.tile([P, TOK], mybir.dt.float32, tag="ps")
                    ps3 = psp.tile([P, TOK], mybir.dt.float32, tag="ps")
                    for d in range(D_T):
                        nc.tensor.matmul(
                            out=ps1[:],
                            lhsT=wa[d][:, hsl],
                            rhs=xt_sb[d][:],
                            start=(d == 0),
                            stop=(d == D_T - 1),
                        )
                    for d in range(D_T):
                        nc.tensor.matmul(
                            out=ps3[:],
                            lhsT=wb[d][:, hsl],
                            rhs=xt_sb[d][:],
                            start=(d == 0),
                            stop=(d == D_T - 1),
                        )
                    sil = tmp.tile([P, TOK], cdt, tag="sil")
                    nc.scalar.activation(
                        out=sil[:], in_=ps1[:], func=mybir.ActivationFunctionType.Silu
                    )
                    nc.vector.tensor_mul(h_sb[:, hg, :], sil[:], ps3[:])

                # ---- phase 2: out[tok, :] = h.T @ w2, dout in two halves ----
                for dh in range(2):
                    dsl = slice(dh * (D // 2), (dh + 1) * (D // 2))
                    pso = [psp.tile([P, TOK], mybir.dt.float32, tag="ps",
                                    name=f"pso_{s}_{dh}_{i}")
                           for i in range(2 * TK)]
                    for hc in range(H_T):
                        for tk in range(TK):
                            lhsT = h_sb[:, hc, tk * P:(tk + 1) * P]
                            for dc in range(2):
                                nc.tensor.matmul(
                                    out=pso[tk * 2 + dc][:],
                                    lhsT=lhsT,
                                    rhs=w2_sb[dh][hc][:, dc * TOK:(dc + 1) * TOK],
                                    start=(hc == 0),
                                    stop=(hc == H_T - 1),
                                )
                    for tk in range(TK):
                        o_sb = outp.tile([P, D // 2], mybir.dt.float32, tag="o")
                        for dc in range(2):
                            nc.vector.tensor_copy(
                                out=o_sb[:, dc * TOK:(dc + 1) * TOK],
                                in_=pso[tk * 2 + dc][:],
                            )
                        nc.gpsimd.dma_start(
                            out=out[s * TOK + tk * P: s * TOK + (tk + 1) * P, dsl],
                            in_=o_sb[:],
                        )
    nc.compile()
    return nc


def _plan(m_sizes, T):
    """Mirror the reference routing: contiguous segments by expert, then chop
    into TOK-sized chunks and deal them contiguously across cores."""
    bounds = np.cumsum(np.asarray(m_sizes, dtype=np.int64))
    E = len(bounds)
    chunks = []  # (expert, row_start, nrows)
    prev = 0
    for e in range(E):
        lo, hi = prev, min(int(bounds[e]), T)
        prev = max(lo, hi)
        seg = hi - lo
        off = lo
        while seg > 0:
            take = min(TOK, seg)
            chunks.append((e, off, take))
            off += take
            seg -= take
    S = max(1, math.ceil(len(chunks) / N_CORES))
    while len(chunks) < N_CORES * S:
        chunks.append((0, 0, 0))  # dummy slot
    per_core = [chunks[c * S:(c + 1) * S] for c in range(N_CORES)]
    return per_core, S


def kernel(x, w1, w2, w3, m_sizes, _trace=False):
    x = np.asarray(x, dtype=np.float32)
    w1 = np.asarray(w1, dtype=np.float32)
    w2 = np.asarray(w2, dtype=np.float32)
    w3 = np.asarray(w3, dtype=np.float32)
    T = x.shape[0]
    assert x.shape[1] == D and w1.shape[1:] == (D, H), (x.shape, w1.shape)
    assert w2.shape[1:] == (H, D) and w3.shape[1:] == (D, H), (w2.shape, w3.shape)

    per_core, S = _plan(m_sizes, T)

    cdt = mybir.dt.bfloat16
    npdt = ml_dtypes.bfloat16

    key = (S, cdt)
    if key not in _compiled_cache:
        _compiled_cache[key] = _build_program(S, cdt)
    nc = _compiled_cache[key]

    w1b = w1.astype(npdt)
    w2b = w2.astype(npdt)
    w3b = w3.astype(npdt)

    in_maps = []
    for c in range(N_CORES):
        slots = per_core[c]
        seg = np.zeros((S * TOK, D), dtype=np.float32)
        for s, (e, off, ln) in enumerate(slots):
            if ln:
                seg[s * TOK:s * TOK + ln] = x[off:off + ln]
        xt_c = np.ascontiguousarray(seg.T).astype(npdt)
        eids = [e for (e, _, _) in slots]
        in_maps.append({
            "xt": xt_c,
            "w1": np.ascontiguousarray(w1b[eids]),
            "w3": np.ascontiguousarray(w3b[eids]),
            "w2": np.ascontiguousarray(w2b[eids]),
        })

    try:
        res = run_bass_kernel_spmd(
            nc, in_maps, list(range(N_CORES)), trace=_trace,
        )
    except Exception:
        # transient NRT device errors have been observed once after a fresh
        # compile; a single retry is free if the device truly died
        res = run_bass_kernel_spmd(
            nc, in_maps, list(range(N_CORES)), trace=_trace,
        )

    full = np.zeros((T, D), dtype=np.float32)
    for c in range(N_CORES):
        oc = res.results[c]["out"]
        for s, (e, off, ln) in enumerate(per_core[c]):
            if ln:
                full[off:off + ln] = oc[s * TOK:s * TOK + ln]

    last_run_info.clear()
    last_run_info.update({
        "exec_time_ns": res.exec_time_ns,
        "profile_json": getattr(res, "profile_json", None),
        "S": S,
    })
    return full



# revision 3
# speedup vs baseline: 1.2781x; 1.2781x over previous
"""Grouped SwiGLU experts (MoE, contiguous per-expert token segments) on 8 trn2 cores.

Strategy: expert-parallel over 512-token slots (as the bf16 baseline), but all
matmuls run in fp8-e4m3 with MatmulPerfMode.DoubleRow (K=256 per instruction,
0.5 cycles/row -> 4x bf16 FLOP rate).  Plain fp8 is ~6.5% rel err, far over
the 2e-2 budget, so every tensor is carried as an exact-ish hi+lo e4m3 pair
and each GEMM computes the three significant cross terms
    a@b ~= a_hi@b_hi + a_hi@b_lo + a_lo@b_hi        (lo@lo ~ 1e-3 rel, dropped)
The lo parts keep the SAME scale as their hi parts, so all three terms
accumulate into a single PSUM bank with no fixup arithmetic (end-to-end rel
err ~3e-3 measured in numpy).  Net tensor-engine time is 0.75x rows vs 1.0x
for bf16 -> ~1.3x speedup.

Scales keep everything in e4m3's happy range: x at 1, w1 at 64, w3 at 16,
w2 at 64; h is produced at scale 16 (= silu(ps1/64) * ps3), output PSUM is
scale 1024 and is written back as bf16 after a 1/1024 scale.

H=1408 is 11 128-chunks (odd), so phase-2 operands carry one zero pad chunk
to keep every matmul a DoubleRow pair: h layout [hh*11, Z, hl*11, Z], w2
layout [wh*11, Z, wl*11, Z]; term T uses (h base, w2 base) offsets
(0,0), (0,12), (12,0).

kernel(**inputs) -> full [16384, 2048] fp32 output.  Self-contained.
"""

import math

import numpy as np
import ml_dtypes

import concourse.bass as bass
import concourse.tile as tile
from concourse import bacc
from concourse import mybir
from concourse.alu_op_type import AluOpType
from concourse.bass_utils import run_bass_kernel_spmd

N_CORES = 8
D = 2048          # dim_in
H = 1408          # dim_hidden
TOK = 512         # tokens per slot
P = 128           # partitions
KC = D // P       # 16 k-chunks over dim_in
HC = H // P       # 11 hid-chunks
TK = TOK // P     # 4 token tiles per slot
NG = 512          # phase-2 out-column group width
NGRP = D // NG    # 4 column groups
NC2 = 2 * (HC + 1)  # 24: phase-2 chunk axis [hi*11, Z, lo*11, Z]
WR = 6            # w13 tag rotation depth (SBUF vs prefetch tradeoff)

SW1 = 64.0        # w1 quant scale
SW3 = 16.0        # w3 quant scale -> h comes out at scale 16
SW2 = 64.0        # w2 quant scale
OSC = 1.0 / (16.0 * 64.0)  # final psum -> out scale

F8 = ml_dtypes.float8_e4m3

_compiled_cache = {}
_wq_cache = {}
last_run_info = {}


def _build_program(S):
    """Per-core SPMD program: S slots, each 512 tokens of one expert."""
    nc = bacc.Bacc()
    f8 = mybir.dt.float8e4
    f32 = mybir.dt.float32
    bf16 = mybir.dt.bfloat16
    DRM = mybir.MatmulPerfMode.DoubleRow
    Act = mybir.ActivationFunctionType

    xt = nc.declare_dram_parameter("xt", [S, 2, P, KC, TOK], f8, isOutput=False)
    w13 = nc.declare_dram_parameter("w13", [S, HC, 4, P, KC, P], f8, isOutput=False)
    w2c = nc.declare_dram_parameter("w2c", [S, NGRP, P, NC2, NG], f8, isOutput=False)
    out = nc.declare_dram_parameter("out", [S * TOK, D], bf16, isOutput=True)

    with tile.TileContext(nc) as tc:
        with (
            tc.tile_pool(name="xp", bufs=2) as xp,
            tc.tile_pool(name="wp", bufs=1) as wp,
            tc.tile_pool(name="w2p", bufs=1) as w2p,
            tc.tile_pool(name="hp", bufs=2) as hp,
            tc.tile_pool(name="tp", bufs=2) as tp,
            tc.tile_pool(name="op", bufs=4) as op,
            tc.tile_pool(name="ps", bufs=8, space="PSUM") as psp,
        ):
            for s in range(S):
                # ---- loads ----
                xh = xp.tile([P, KC, TOK], f8, tag="xh", name=f"xh_{s}")
                nc.sync.dma_start(out=xh[:], in_=xt[s, 0])
                xl = xp.tile([P, KC, TOK], f8, tag="xl", name=f"xl_{s}")
                nc.sync.dma_start(out=xl[:], in_=xt[s, 1])
                wt = {}
                for hg in range(HC):
                    for k in range(4):
                        t = wp.tile([P, KC, P], f8, tag=f"w_{hg % WR}_{k}",
                                    name=f"w13_{s}_{hg}_{k}")
                        nc.sync.dma_start(out=t[:], in_=w13[s, hg, k])
                        wt[(hg, k)] = t
                w2t = []
                for g in range(NGRP):
                    t = w2p.tile([P, NC2, NG], f8, tag=f"w2_{g}",
                                 name=f"w2_{s}_{g}")
                    nc.sync.dma_start(out=t[:], in_=w2c[s, g])
                    w2t.append(t)

                h = hp.tile([P, NC2, TOK], f8, tag="h", name=f"h_{s}")
                # zero pad chunks (positions HC and NC2-1) for DoubleRow pairing
                nc.gpsimd.memset(h[:, HC, :], 0)
                nc.gpsimd.memset(h[:, NC2 - 1, :], 0)

                # ---- phase 1: h = silu(x@w1) * (x@w3), hi/lo split on chip ----
                for hg in range(HC):
                    ps1 = psp.tile([P, TOK], f32, tag="ps", name=f"ps1_{s}_{hg}")
                    ps3 = psp.tile([P, TOK], f32, tag="ps", name=f"ps3_{s}_{hg}")
                    for psx, khi, klo in ((ps1, 0, 1), (ps3, 2, 3)):
                        n = 0
                        for wk, xx in ((khi, xh), (klo, xh), (khi, xl)):
                            w = wt[(hg, wk)]
                            for d in range(KC // 2):
                                nc.tensor.matmul(
                                    out=psx[:],
                                    lhsT=w[:, 2 * d:2 * d + 2, :],
                                    rhs=xx[:, 2 * d:2 * d + 2, :],
                                    start=(n == 0),
                                    stop=(n == 3 * (KC // 2) - 1),
                                    perf_mode=DRM,
                                )
                                n += 1
                    sil = tp.tile([P, TOK], f32, tag="sil", name=f"sil_{s}_{hg}")
                    nc.scalar.activation(sil[:], ps1[:], Act.Silu, scale=1.0 / SW1)
                    h16 = tp.tile([P, TOK], f32, tag="h16", name=f"h16_{s}_{hg}")
                    nc.vector.tensor_tensor(out=h16[:], in0=sil[:], in1=ps3[:],
                                            op=AluOpType.mult)
                    nc.gpsimd.tensor_copy(out=h[:, hg, :], in_=h16[:])
                    nc.vector.tensor_tensor(out=h[:, HC + 1 + hg, :], in0=h16[:],
                                            in1=h[:, hg, :], op=AluOpType.subtract)

                # ---- phase 2: out = (h_hi+h_lo) @ (w2_hi+w2_lo), 3 terms ----
                for g in range(NGRP):
                    for tk in range(TK):
                        pso = psp.tile([P, NG], f32, tag="ps",
                                       name=f"pso_{s}_{g}_{tk}")
                        n = 0
                        for hbase, wbase in ((0, 0), (0, HC + 1), (HC + 1, 0)):
                            for c in range((HC + 1) // 2):
                                nc.tensor.matmul(
                                    out=pso[:],
                                    lhsT=h[:, hbase + 2 * c:hbase + 2 * c + 2,
                                           tk * P:(tk + 1) * P],
                                    rhs=w2t[g][:, wbase + 2 * c:wbase + 2 * c + 2, :],
                                    start=(n == 0),
                                    stop=(n == 3 * (HC + 1) // 2 - 1),
                                    perf_mode=DRM,
                                )
                                n += 1
                        o = op.tile([P, NG], bf16, tag="o", name=f"o_{s}_{g}_{tk}")
                        if (g * TK + tk) % 2 == 0:
                            nc.vector.tensor_scalar_mul(out=o[:], in0=pso[:],
                                                        scalar1=OSC)
                        else:
                            nc.scalar.activation(o[:], pso[:], Act.Copy, scale=OSC)
                        nc.gpsimd.dma_start(
                            out=out[s * TOK + tk * P:s * TOK + (tk + 1) * P,
                                    g * NG:(g + 1) * NG],
                            in_=o[:],
                        )
    nc.compile()
    return nc


def _plan(m_sizes, T):
    """Mirror the reference routing: contiguous segments by expert, then chop
    into TOK-sized chunks and deal them contiguously across cores."""
    bounds = np.cumsum(np.asarray(m_sizes, dtype=np.int64))
    E = len(bounds)
    chunks = []  # (expert, row_start, nrows)
    prev = 0
    for e in range(E):
        lo, hi = prev, min(int(bounds[e]), T)
        prev = max(lo, hi)
        seg = hi - lo
        off = lo
        while seg > 0:
            take = min(TOK, seg)
            chunks.append((e, off, take))
            off += take
            seg -= take
    S = max(1, math.ceil(len(chunks) / N_CORES))
    while len(chunks) < N_CORES * S:
        chunks.append((0, 0, 0))  # dummy slot
    per_core = [chunks[c * S:(c + 1) * S] for c in range(N_CORES)]
    return per_core, S


def _hilo(a):
    hi = a.astype(F8)
    lo = (a - hi.astype(np.float32)).astype(F8)
    return hi, lo


def _quant_weights(w1, w2, w3):
    """Per-expert hi/lo fp8 weights in the on-device layouts."""
    E = w1.shape[0]
    w13_e = np.empty((E, HC, 4, P, KC, P), dtype=F8)
    w2_e = np.zeros((E, NGRP, P, NC2, NG), dtype=F8)

    def t13(a):  # [D, H] -> [HC, P(k), KC, P(h)]
        return a.reshape(KC, P, HC, P).transpose(2, 1, 0, 3)

    def t2(a):  # [H, D] -> [NGRP, P(h), HC, NG]
        return a.reshape(HC, P, NGRP, NG).transpose(2, 1, 0, 3)

    for e in range(E):
        h1, l1 = _hilo(w1[e] * SW1)
        h3, l3 = _hilo(w3[e] * SW3)
        w13_e[e, :, 0] = t13(h1)
        w13_e[e, :, 1] = t13(l1)
        w13_e[e, :, 2] = t13(h3)
        w13_e[e, :, 3] = t13(l3)
        h2, l2 = _hilo(w2[e] * SW2)
        w2_e[e, :, :, 0:HC] = t2(h2)
        w2_e[e, :, :, HC + 1:NC2 - 1] = t2(l2)
    return w13_e, w2_e


def kernel(x, w1, w2, w3, m_sizes, _trace=False):
    x = np.asarray(x, dtype=np.float32)
    w1 = np.asarray(w1, dtype=np.float32)
    w2 = np.asarray(w2, dtype=np.float32)
    w3 = np.asarray(w3, dtype=np.float32)
    T = x.shape[0]
    assert x.shape[1] == D and w1.shape[1:] == (D, H), (x.shape, w1.shape)
    assert w2.shape[1:] == (H, D) and w3.shape[1:] == (D, H), (w2.shape, w3.shape)

    per_core, S = _plan(m_sizes, T)

    if S not in _compiled_cache:
        _compiled_cache[S] = _build_program(S)
    nc = _compiled_cache[S]

    wkey = (id(w1), id(w2), id(w3))
    if wkey not in _wq_cache:
        _wq_cache.clear()
        _wq_cache[wkey] = _quant_weights(w1, w2, w3)
    w13_e, w2_e = _wq_cache[wkey]

    in_maps = []
    for c in range(N_CORES):
        slots = per_core[c]
        xt_c = np.zeros((S, 2, P, KC, TOK), dtype=F8)
        for s, (e, off, ln) in enumerate(slots):
            if ln:
                seg = np.zeros((TOK, D), dtype=np.float32)
                seg[:ln] = x[off:off + ln]
                sh, sl = _hilo(seg)
                # [TOK, D] -> [P(k), KC, TOK]
                xt_c[s, 0] = sh.reshape(TOK, KC, P).transpose(2, 1, 0)
                xt_c[s, 1] = sl.reshape(TOK, KC, P).transpose(2, 1, 0)
        eids = [e for (e, _, _) in slots]
        in_maps.append({
            "xt": xt_c,
            "w13": np.ascontiguousarray(w13_e[eids]),
            "w2c": np.ascontiguousarray(w2_e[eids]),
        })

    try:
        res = run_bass_kernel_spmd(
            nc, in_maps, list(range(N_CORES)), trace=_trace,
        )
    except Exception:
        # transient NRT device errors have been observed once after a fresh
        # compile; a single retry is free if the device truly died
        res = run_bass_kernel_spmd(
            nc, in_maps, list(range(N_CORES)), trace=_trace,
        )

    full = np.zeros((T, D), dtype=np.float32)
    for c in range(N_CORES):
        oc = res.results[c]["out"].astype(np.float32)
        for s, (e, off, ln) in enumerate(per_core[c]):
            if ln:
                full[off:off + ln] = oc[s * TOK:s * TOK + ln]

    last_run_info.clear()
    last_run_info.update({
        "exec_time_ns": res.exec_time_ns,
        "profile_json": getattr(res, "profile_json", None),
        "S": S,
    })
    return full


# revision 29
# speedup vs baseline: 1.3195x; 1.0323x over previous
"""Grouped SwiGLU experts (MoE, contiguous per-expert token segments) on 8 trn2 cores.

Strategy: expert-parallel over 512-token slots (as the bf16 baseline), but all
matmuls run in fp8-e4m3 with MatmulPerfMode.DoubleRow (K=256 per instruction,
0.5 cycles/row -> 4x bf16 FLOP rate).  Plain fp8 is ~6.5% rel err, far over
the 2e-2 budget, so every tensor is carried as an exact-ish hi+lo e4m3 pair
and each GEMM computes the three significant cross terms
    a@b ~= a_hi@b_hi + a_hi@b_lo + a_lo@b_hi        (lo@lo ~ 1e-3 rel, dropped)
The lo parts keep the SAME scale as their hi parts, so all three terms
accumulate into a single PSUM bank with no fixup arithmetic (end-to-end rel
err ~3e-3 measured in numpy).  Net tensor-engine time is 0.75x rows vs 1.0x
for bf16 -> ~1.3x speedup.

Scales keep everything in e4m3's happy range: x at 1, w1 at 64, w3 at 16,
w2 at 64; h is produced at scale 16 (= silu(ps1/64) * ps3), output PSUM is
scale 1024 and is written back as bf16 after a 1/1024 scale.

H=1408 is 11 128-chunks (odd = 33 product chunks over the 3 terms), so the
phase-2 chunk layouts are arranged to cover all 33 in exactly 17 DoubleRow
pairs: h is [hh0..hh10, hh10(dup), hl0..hl10, Z] and w2 is
[wh0..wh10, wl10, wl0..wl9].  Pairs:
  5x (hh2c,hh2c+1)x(wh2c,wh2c+1)   main
  1x (hh10,hh10')x(wh10,wl10)      both hh10 leftovers in one pair
  5x (hh2c,hh2c+1)x(wl2c,wl2c+1)   w2-lo correction
  5x (hl2c,hl2c+1)x(wh2c,wh2c+1)   h-lo correction
  1x (hl10,Z)x(wh10,wl10)          last h-lo leftover

kernel(**inputs) -> full [16384, 2048] fp32 output.  Self-contained.
"""

import math

import numpy as np
import ml_dtypes

import concourse.bass as bass
import concourse.tile as tile
from concourse import bacc
from concourse import mybir
from concourse.alu_op_type import AluOpType
from concourse.bass_utils import run_bass_kernel_spmd

N_CORES = 8
D = 2048          # dim_in
H = 1408          # dim_hidden
TOK = 512         # tokens per slot
P = 128           # partitions
KC = D // P       # 16 k-chunks over dim_in
HC = H // P       # 11 hid-chunks
TK = TOK // P     # 4 token tiles per slot
NG = 512          # phase-2 out-column group width
NGRP = D // NG    # 4 column groups
NCH = 24          # h chunk axis: [hh*11, hh10dup, hl*11, Z]
NCW = 22          # w2 chunk axis: [wh*11, wl10, wl0..wl9]
WR = 5            # w13 tag rotation depth (SBUF vs prefetch tradeoff)

SW1 = 64.0        # w1 quant scale
SW3 = 16.0        # w3 quant scale -> h comes out at scale 16
SW2 = 64.0        # w2 quant scale
OSC = 1.0 / (16.0 * 64.0)  # final psum -> out scale

F8 = ml_dtypes.float8_e4m3

_compiled_cache = {}
_wq_cache = {}
last_run_info = {}


def _build_program(S):
    """Per-core SPMD program: S slots, each 512 tokens of one expert."""
    nc = bacc.Bacc()
    f8 = mybir.dt.float8e4
    f32 = mybir.dt.float32
    bf16 = mybir.dt.bfloat16
    DRM = mybir.MatmulPerfMode.DoubleRow
    Act = mybir.ActivationFunctionType

    xt = nc.declare_dram_parameter("xt", [S, 2, P, KC, TOK], f8, isOutput=False)
    w13 = nc.declare_dram_parameter("w13", [S, HC, P, 4, KC, P], f8, isOutput=False)
    w2c = nc.declare_dram_parameter("w2c", [S, NGRP, P, NCW, NG], f8, isOutput=False)
    out = nc.declare_dram_parameter("out", [S * TOK, D], bf16, isOutput=True)

    with tile.TileContext(nc) as tc:
        with (
            tc.tile_pool(name="xp", bufs=2) as xp,
            tc.tile_pool(name="wp", bufs=1) as wp,
            tc.tile_pool(name="w2p", bufs=1) as w2p,
            tc.tile_pool(name="hp", bufs=2) as hp,
            tc.tile_pool(name="tp", bufs=2) as tp,
            tc.tile_pool(name="op", bufs=4) as op,
            tc.tile_pool(name="psA", bufs=5, space="PSUM") as psa,
            tc.tile_pool(name="psB", bufs=3, space="PSUM") as psb,
        ):
            # phase-1 processing order: hg10 runs mid-phase so its h chunks
            # (hh10, the dup, hl10) are long done before phase 2 reads them
            # in its final DoubleRow pairs
            PROC = [0, 1, 2, 3, 4, 10, 5, 6, 7, 8, 9]

            # PE pstate warmup: ~150 tiny self-contained matmuls on scratch
            # data keep the PE busy through the initial DMA wait so the
            # first real chains run at full clock
            scr_w = tp.tile([P, 2, P], f8, tag="scrw", bufs=1, name="scr_w")
            scr_x = tp.tile([P, 2, 64], f8, tag="scrx", bufs=1, name="scr_x")
            nc.gpsimd.memset(scr_w[:], 0)
            nc.gpsimd.memset(scr_x[:], 0)
            scr_ps = psa.tile([P, TOK], f32, tag="ps", name="scr_ps")
            for i in range(150):
                nc.tensor.matmul(out=scr_ps[:, 0:64], lhsT=scr_w[:], rhs=scr_x[:],
                                 start=True, stop=True, perf_mode=DRM)

            for s in range(S):
                # ---- loads.  All on the SP queue, which is a strict
                # blocking FIFO (a DMA holds the SEQ during its semaphore
                # waits), so issue order IS priority order.  w13 issues are
                # paced by their tag-rotation frees; by the time the FIFO
                # reaches this slot's w2 issues, the previous slot's phase 2
                # is done, so they never block later loads. ----
                xh = xp.tile([P, KC, TOK], f8, tag="xh", name=f"xh_{s}")
                xl = xp.tile([P, KC, TOK], f8, tag="xl", name=f"xl_{s}")
                wt = {}

                def load_w13(pos, s=s, wt=wt):
                    hg = PROC[pos]
                    t = wp.tile([P, 4, KC, P], f8, tag=f"w_{pos % WR}",
                                name=f"w13_{s}_{hg}")
                    nc.sync.dma_start(out=t[:], in_=w13[s, hg])
                    wt[hg] = t

                KH = KC // 2
                if s == 0:
                    # fine-grained first loads: the first chain can start
                    # after one x quarter + half a w13 kind instead of 2MB
                    t = wp.tile([P, 4, KC, P], f8, tag="w_0", name="w13_0_0")
                    wt[0] = t
                    nc.sync.dma_start(out=t[:, 0, 0:KH, :], in_=w13[0, 0, :, 0, 0:KH])
                    nc.sync.dma_start(out=xh[:, 0:4, :], in_=xt[0, 0, :, 0:4])
                    nc.sync.dma_start(out=t[:, 0, KH:KC, :], in_=w13[0, 0, :, 0, KH:KC])
                    nc.sync.dma_start(out=xh[:, 4:8, :], in_=xt[0, 0, :, 4:8])
                    nc.sync.dma_start(out=t[:, 1], in_=w13[0, 0, :, 1])
                    nc.sync.dma_start(out=xh[:, 8:12, :], in_=xt[0, 0, :, 8:12])
                    nc.sync.dma_start(out=xh[:, 12:16, :], in_=xt[0, 0, :, 12:16])
                    nc.sync.dma_start(out=xl[:, 0:KH, :], in_=xt[0, 1, :, 0:KH])
                    nc.sync.dma_start(out=t[:, 2], in_=w13[0, 0, :, 2])
                    nc.sync.dma_start(out=t[:, 3], in_=w13[0, 0, :, 3])
                    nc.sync.dma_start(out=xl[:, KH:KC, :], in_=xt[0, 1, :, KH:KC])
                else:
                    nc.sync.dma_start(out=xh[:, 0:KH, :], in_=xt[s, 0, :, 0:KH])
                    load_w13(0)
                    nc.sync.dma_start(out=xh[:, KH:KC, :], in_=xt[s, 0, :, KH:KC])
                    nc.sync.dma_start(out=xl[:, 0:KH, :], in_=xt[s, 1, :, 0:KH])
                    nc.sync.dma_start(out=xl[:, KH:KC, :], in_=xt[s, 1, :, KH:KC])
                for pos in range(1, HC):
                    load_w13(pos)
                w2t = []
                for g in range(NGRP):
                    t = w2p.tile([P, NCW, NG], f8, tag=f"w2_{g}",
                                 name=f"w2_{s}_{g}")
                    nc.sync.dma_start(out=t[:], in_=w2c[s, g])
                    w2t.append(t)

                h = hp.tile([P, NCH, TOK], f8, tag="h", name=f"h_{s}")
                # zero pad chunk at the end for the last h-lo DoubleRow pair
                nc.gpsimd.memset(h[:, NCH - 1, :], 0)

                # ---- phase 1: h = silu(x@w1) * (x@w3), hi/lo split on chip ----
                for hg in PROC:
                    ps1 = psa.tile([P, TOK], f32, tag="ps", name=f"ps1_{s}_{hg}")
                    ps3 = psa.tile([P, TOK], f32, tag="ps", name=f"ps3_{s}_{hg}")
                    w = wt[hg]
                    for psx, khi, klo in ((ps1, 0, 1), (ps3, 2, 3)):
                        n = 0
                        for wk, xx in ((khi, xh), (klo, xh), (khi, xl)):
                            for d in range(KC // 2):
                                nc.tensor.matmul(
                                    out=psx[:],
                                    lhsT=w[:, wk, 2 * d:2 * d + 2, :],
                                    rhs=xx[:, 2 * d:2 * d + 2, :],
                                    start=(n == 0),
                                    stop=(n == 3 * (KC // 2) - 1),
                                    perf_mode=DRM,
                                )
                                n += 1
                    sil = tp.tile([P, TOK], f32, tag="sil", name=f"sil_{s}_{hg}")
                    nc.scalar.activation(sil[:], ps1[:], Act.Silu, scale=1.0 / SW1)
                    h16 = tp.tile([P, TOK], f32, tag="h16", name=f"h16_{s}_{hg}")
                    nc.vector.tensor_tensor(out=h16[:], in0=sil[:], in1=ps3[:],
                                            op=AluOpType.mult)
                    nc.gpsimd.tensor_copy(out=h[:, hg, :], in_=h16[:])
                    if hg == HC - 1:  # duplicate hh10 for the leftover pair
                        nc.gpsimd.tensor_copy(out=h[:, HC, :], in_=h16[:])
                    nc.vector.tensor_tensor(out=h[:, HC + 1 + hg, :], in0=h16[:],
                                            in1=h[:, hg, :], op=AluOpType.subtract)

                # ---- phase 2: out = (h_hi+h_lo) @ (w2_hi+w2_lo), 3 terms
                # covered by 17 DoubleRow pairs (see module docstring) ----
                HL = HC + 1  # h-lo chunk base (12)
                WL = HC + 1  # w2-lo chunk base (12); wl10 sits at 11
                # pair order: the 14 pairs whose h chunks are written by
                # mid-phase-1 come first; the 3 pairs touching the last
                # processed hid-chunk's writes (hh8/hh9 cast, hl8/hl9 sub)
                # come last, so phase 2 can start before phase 1's tail
                # elementwise ops land
                p2_pairs = (
                    [(2 * c, 2 * c) for c in range(4)]            # main c0-3
                    + [(2 * c, WL + 2 * c) for c in range(4)]     # w2-lo c0-3
                    + [(HL + 2 * c, 2 * c) for c in range(4)]     # h-lo c0-3
                    + [(HC - 1, HC - 1)]                          # hh10 x (wh10,wl10)
                    + [(HL + HC - 1, HC - 1)]                     # (hl10,Z) x (wh10,wl10)
                    + [(8, 8), (8, WL + 8)]                       # main/w2-lo c4 (hh9)
                    + [(HL + 8, 8)]                               # h-lo c4 (hl9)
                )
                NEARLY = 14  # pairs with no dependency on the last hg's writes

                def p2_chain(ci, lo, hi, pso):
                    g, tk = ci // TK, ci % TK
                    for n in range(lo, hi):
                        hc, wc = p2_pairs[n]
                        nc.tensor.matmul(
                            out=pso[:],
                            lhsT=h[:, hc:hc + 2, tk * P:(tk + 1) * P],
                            rhs=w2t[g][:, wc:wc + 2, :],
                            start=(n == 0),
                            stop=(n == len(p2_pairs) - 1),
                            perf_mode=DRM,
                        )

                def p2_finish(ci, pso, o_sb):
                    g, tk = ci // TK, ci % TK
                    if g % 2 == 0:
                        o_sb[tk] = op.tile([P, 2 * NG], bf16, tag=f"o_{tk}",
                                           bufs=2, name=f"o_{s}_{g // 2}_{tk}")
                    o = o_sb[tk]
                    half = o[:, (g % 2) * NG:(g % 2 + 1) * NG]
                    if ci % 2 == 0:
                        nc.vector.tensor_scalar_mul(out=half, in0=pso[:],
                                                    scalar1=OSC)
                    else:
                        nc.scalar.activation(half, pso[:], Act.Copy, scale=OSC)
                    if g % 2 == 1:
                        nc.gpsimd.dma_start(
                            out=out[s * TOK + tk * P:s * TOK + (tk + 1) * P,
                                    (g // 2) * 2 * NG:(g // 2 + 1) * 2 * NG],
                            in_=o[:],
                        )

                o_sb = {}
                NP2 = len(p2_pairs)
                # first three chains interleaved: their early pairs run while
                # the last hg's h writes land, then their tails complete
                first = [psb.tile([P, NG], f32, tag="ps", name=f"pso_{s}_{ci}")
                         for ci in range(3)]
                for ci in range(3):
                    p2_chain(ci, 0, NEARLY, first[ci])
                for ci in range(3):
                    p2_chain(ci, NEARLY, NP2, first[ci])
                    p2_finish(ci, first[ci], o_sb)
                for ci in range(3, NGRP * TK):
                    pso = psb.tile([P, NG], f32, tag="ps", name=f"pso_{s}_{ci}")
                    p2_chain(ci, 0, NP2, pso)
                    p2_finish(ci, pso, o_sb)
    nc.compile()
    return nc


def _plan(m_sizes, T):
    """Mirror the reference routing: contiguous segments by expert, then chop
    into TOK-sized chunks and deal them contiguously across cores."""
    bounds = np.cumsum(np.asarray(m_sizes, dtype=np.int64))
    E = len(bounds)
    chunks = []  # (expert, row_start, nrows)
    prev = 0
    for e in range(E):
        lo, hi = prev, min(int(bounds[e]), T)
        prev = max(lo, hi)
        seg = hi - lo
        off = lo
        while seg > 0:
            take = min(TOK, seg)
            chunks.append((e, off, take))
            off += take
            seg -= take
    S = max(1, math.ceil(len(chunks) / N_CORES))
    while len(chunks) < N_CORES * S:
        chunks.append((0, 0, 0))  # dummy slot
    per_core = [chunks[c * S:(c + 1) * S] for c in range(N_CORES)]
    return per_core, S


def _hilo(a):
    hi = a.astype(F8)
    lo = (a - hi.astype(np.float32)).astype(F8)
    return hi, lo


def _quant_weights(w1, w2, w3):
    """Per-expert hi/lo fp8 weights in the on-device layouts."""
    E = w1.shape[0]
    w13_e = np.empty((E, HC, P, 4, KC, P), dtype=F8)
    w2_e = np.empty((E, NGRP, P, NCW, NG), dtype=F8)

    def t13(a):  # [D, H] -> [HC, P(k), KC, P(h)]
        return a.reshape(KC, P, HC, P).transpose(2, 1, 0, 3)

    def t2(a):  # [H, D] -> [NGRP, P(h), HC, NG]
        return a.reshape(HC, P, NGRP, NG).transpose(2, 1, 0, 3)

    for e in range(E):
        h1, l1 = _hilo(w1[e] * SW1)
        h3, l3 = _hilo(w3[e] * SW3)
        w13_e[e, :, :, 0] = t13(h1)
        w13_e[e, :, :, 1] = t13(l1)
        w13_e[e, :, :, 2] = t13(h3)
        w13_e[e, :, :, 3] = t13(l3)
        h2, l2 = _hilo(w2[e] * SW2)
        th, tl = t2(h2), t2(l2)
        w2_e[e, :, :, 0:HC] = th          # wh0..wh10
        w2_e[e, :, :, HC] = tl[:, :, HC - 1]   # wl10
        w2_e[e, :, :, HC + 1:NCW] = tl[:, :, 0:HC - 1]  # wl0..wl9
    return w13_e, w2_e


def kernel(x, w1, w2, w3, m_sizes, _trace=False):
    x = np.asarray(x, dtype=np.float32)
    w1 = np.asarray(w1, dtype=np.float32)
    w2 = np.asarray(w2, dtype=np.float32)
    w3 = np.asarray(w3, dtype=np.float32)
    T = x.shape[0]
    assert x.shape[1] == D and w1.shape[1:] == (D, H), (x.shape, w1.shape)
    assert w2.shape[1:] == (H, D) and w3.shape[1:] == (D, H), (w2.shape, w3.shape)

    per_core, S = _plan(m_sizes, T)

    if S not in _compiled_cache:
        _compiled_cache[S] = _build_program(S)
    nc = _compiled_cache[S]

    wkey = (id(w1), id(w2), id(w3))
    if wkey not in _wq_cache:
        _wq_cache.clear()
        _wq_cache[wkey] = _quant_weights(w1, w2, w3)
    w13_e, w2_e = _wq_cache[wkey]

    in_maps = []
    for c in range(N_CORES):
        slots = per_core[c]
        xt_c = np.zeros((S, 2, P, KC, TOK), dtype=F8)
        for s, (e, off, ln) in enumerate(slots):
            if ln:
                seg = np.zeros((TOK, D), dtype=np.float32)
                seg[:ln] = x[off:off + ln]
                sh, sl = _hilo(seg)
                # [TOK, D] -> [P(k), KC, TOK]
                xt_c[s, 0] = sh.reshape(TOK, KC, P).transpose(2, 1, 0)
                xt_c[s, 1] = sl.reshape(TOK, KC, P).transpose(2, 1, 0)
        eids = [e for (e, _, _) in slots]
        in_maps.append({
            "xt": xt_c,
            "w13": np.ascontiguousarray(w13_e[eids]),
            "w2c": np.ascontiguousarray(w2_e[eids]),
        })

    try:
        res = run_bass_kernel_spmd(
            nc, in_maps, list(range(N_CORES)), trace=_trace,
        )
    except Exception:
        # transient NRT device errors have been observed once after a fresh
        # compile; a single retry is free if the device truly died
        res = run_bass_kernel_spmd(
            nc, in_maps, list(range(N_CORES)), trace=_trace,
        )

    full = np.zeros((T, D), dtype=np.float32)
    for c in range(N_CORES):
        oc = res.results[c]["out"].astype(np.float32)
        for s, (e, off, ln) in enumerate(per_core[c]):
            if ln:
                full[off:off + ln] = oc[s * TOK:s * TOK + ln]

    last_run_info.clear()
    last_run_info.update({
        "exec_time_ns": res.exec_time_ns,
        "profile_json": getattr(res, "profile_json", None),
        "S": S,
    })
    return full


# revision 32
# speedup vs baseline: 1.3810x; 1.0466x over previous
"""Grouped SwiGLU experts (MoE, contiguous per-expert token segments) on 8 trn2 cores.

Strategy: expert-parallel over 512-token slots (as the bf16 baseline), but all
matmuls run in fp8-e4m3 with MatmulPerfMode.DoubleRow (K=256 per instruction,
0.5 cycles/row -> 4x bf16 FLOP rate).  Plain fp8 is ~6.5% rel err, far over
the 2e-2 budget, so every tensor is carried as an exact-ish hi+lo e4m3 pair
and each GEMM computes the three significant cross terms
    a@b ~= a_hi@b_hi + a_hi@b_lo + a_lo@b_hi        (lo@lo ~ 1e-3 rel, dropped)
The lo parts keep the SAME scale as their hi parts, so all three terms
accumulate into a single PSUM bank with no fixup arithmetic (end-to-end rel
err ~3e-3 measured in numpy).  Net tensor-engine time is 0.75x rows vs 1.0x
for bf16 -> ~1.3x speedup.

Scales keep everything in e4m3's happy range: x at 1, w1 at 64, w3 at 16,
w2 at 64; h is produced at scale 16 (= silu(ps1/64) * ps3), output PSUM is
scale 1024 and is written back as bf16 after a 1/1024 scale.

H=1408 is 11 128-chunks (odd = 33 product chunks over the 3 terms), so the
phase-2 chunk layouts are arranged to cover all 33 in exactly 17 DoubleRow
pairs: h is [hh0..hh10, hh10(dup), hl0..hl10, Z] and w2 is
[wh0..wh10, wl10, wl0..wl9].  Pairs:
  5x (hh2c,hh2c+1)x(wh2c,wh2c+1)   main
  1x (hh10,hh10')x(wh10,wl10)      both hh10 leftovers in one pair
  5x (hh2c,hh2c+1)x(wl2c,wl2c+1)   w2-lo correction
  5x (hl2c,hl2c+1)x(wh2c,wh2c+1)   h-lo correction
  1x (hl10,Z)x(wh10,wl10)          last h-lo leftover

kernel(**inputs) -> full [16384, 2048] fp32 output.  Self-contained.
"""

import math

import numpy as np
import ml_dtypes

import concourse.bass as bass
import concourse.tile as tile
from concourse import bacc
from concourse import mybir
from concourse.alu_op_type import AluOpType
from concourse.bass_utils import run_bass_kernel_spmd

N_CORES = 8
D = 2048          # dim_in
H = 1408          # dim_hidden
TOK = 512         # tokens per slot
P = 128           # partitions
KC = D // P       # 16 k-chunks over dim_in
HC = H // P       # 11 hid-chunks
TK = TOK // P     # 4 token tiles per slot
NG = 512          # phase-2 out-column group width
NGRP = D // NG    # 4 column groups
NCH = 24          # h chunk axis: [hh*11, hh10dup, hl*11, Z]
NCW = 22          # w2 chunk axis: [wh*11, wl10, wl0..wl9]
WR = 5            # w13 tag rotation depth (SBUF vs prefetch tradeoff)

SW1 = 64.0        # w1 quant scale
SW3 = 16.0        # w3 quant scale -> h comes out at scale 16
SW2 = 64.0        # w2 quant scale
OSC = 1.0 / (16.0 * 64.0)  # final psum -> out scale

F8 = ml_dtypes.float8_e4m3

_compiled_cache = {}
_wq_cache = {}
last_run_info = {}


def _build_program(S):
    """Per-core SPMD program: S slots, each 512 tokens of one expert."""
    nc = bacc.Bacc()
    f8 = mybir.dt.float8e4
    f32 = mybir.dt.float32
    bf16 = mybir.dt.bfloat16
    DRM = mybir.MatmulPerfMode.DoubleRow
    Act = mybir.ActivationFunctionType

    xt = nc.declare_dram_parameter("xt", [S, 2, P, KC, TOK], f8, isOutput=False)
    w13 = nc.declare_dram_parameter("w13", [S, HC, P, 4, KC, P], f8, isOutput=False)
    w2c = nc.declare_dram_parameter("w2c", [S, NGRP, P, NCW, NG], f8, isOutput=False)
    out = nc.declare_dram_parameter("out", [S * TOK, D], bf16, isOutput=True)

    with tile.TileContext(nc) as tc:
        with (
            tc.tile_pool(name="xp", bufs=2) as xp,
            tc.tile_pool(name="wp", bufs=1) as wp,
            tc.tile_pool(name="w2p", bufs=1) as w2p,
            tc.tile_pool(name="hp", bufs=2) as hp,
            tc.tile_pool(name="tp", bufs=2) as tp,
            tc.tile_pool(name="op", bufs=4) as op,
            tc.tile_pool(name="psA", bufs=5, space="PSUM") as psa,
            tc.tile_pool(name="psB", bufs=3, space="PSUM") as psb,
        ):
            # phase-1 processing order: hg10 runs mid-phase so its h chunks
            # (hh10, the dup, hl10) are long done before phase 2 reads them
            # in its final DoubleRow pairs
            PROC = [0, 1, 2, 3, 4, 10, 5, 6, 7, 8, 9]

            # PE pstate warmup: ~150 tiny self-contained matmuls on scratch
            # data keep the PE busy through the initial DMA wait so the
            # first real chains run at full clock
            scr_w = tp.tile([P, 2, P], f8, tag="scrw", bufs=1, name="scr_w")
            scr_x = tp.tile([P, 2, 64], f8, tag="scrx", bufs=1, name="scr_x")
            nc.gpsimd.memset(scr_w[:], 0)
            nc.gpsimd.memset(scr_x[:], 0)
            scr_ps = psa.tile([P, TOK], f32, tag="ps", name="scr_ps")
            for i in range(150):
                nc.tensor.matmul(out=scr_ps[:, 0:64], lhsT=scr_w[:], rhs=scr_x[:],
                                 start=True, stop=True, perf_mode=DRM)

            for s in range(S):
                # ---- loads.  All on the SP queue, which is a strict
                # blocking FIFO (a DMA holds the SEQ during its semaphore
                # waits), so issue order IS priority order.  w13 issues are
                # paced by their tag-rotation frees; by the time the FIFO
                # reaches this slot's w2 issues, the previous slot's phase 2
                # is done, so they never block later loads. ----
                xh = xp.tile([P, KC, TOK], f8, tag="xh", name=f"xh_{s}")
                xl = xp.tile([P, KC, TOK], f8, tag="xl", name=f"xl_{s}")
                wt = {}

                def load_w13(pos, s=s, wt=wt):
                    hg = PROC[pos]
                    t = wp.tile([P, 4, KC, P], f8, tag=f"w_{pos % WR}",
                                name=f"w13_{s}_{hg}")
                    nc.sync.dma_start(out=t[:], in_=w13[s, hg])
                    wt[hg] = t

                KH = KC // 2
                if s == 0:
                    # fine-grained first loads: the first chain can start
                    # after one x quarter + half a w13 kind instead of 2MB
                    t = wp.tile([P, 4, KC, P], f8, tag="w_0", name="w13_0_0")
                    wt[0] = t
                    nc.sync.dma_start(out=t[:, 0, 0:KH, :], in_=w13[0, 0, :, 0, 0:KH])
                    nc.sync.dma_start(out=xh[:, 0:4, :], in_=xt[0, 0, :, 0:4])
                    nc.sync.dma_start(out=t[:, 0, KH:KC, :], in_=w13[0, 0, :, 0, KH:KC])
                    nc.sync.dma_start(out=xh[:, 4:8, :], in_=xt[0, 0, :, 4:8])
                    nc.sync.dma_start(out=t[:, 1], in_=w13[0, 0, :, 1])
                    nc.sync.dma_start(out=xh[:, 8:12, :], in_=xt[0, 0, :, 8:12])
                    nc.sync.dma_start(out=xh[:, 12:16, :], in_=xt[0, 0, :, 12:16])
                    nc.sync.dma_start(out=xl[:, 0:KH, :], in_=xt[0, 1, :, 0:KH])
                    nc.sync.dma_start(out=t[:, 2], in_=w13[0, 0, :, 2])
                    nc.sync.dma_start(out=t[:, 3], in_=w13[0, 0, :, 3])
                    nc.sync.dma_start(out=xl[:, KH:KC, :], in_=xt[0, 1, :, KH:KC])
                else:
                    nc.sync.dma_start(out=xh[:, 0:KH, :], in_=xt[s, 0, :, 0:KH])
                    load_w13(0)
                    nc.sync.dma_start(out=xh[:, KH:KC, :], in_=xt[s, 0, :, KH:KC])
                    nc.sync.dma_start(out=xl[:, 0:KH, :], in_=xt[s, 1, :, 0:KH])
                    nc.sync.dma_start(out=xl[:, KH:KC, :], in_=xt[s, 1, :, KH:KC])
                for pos in range(1, HC):
                    load_w13(pos)
                w2t = []
                for g in range(NGRP):
                    t = w2p.tile([P, NCW, NG], f8, tag=f"w2_{g}",
                                 name=f"w2_{s}_{g}")
                    nc.sync.dma_start(out=t[:], in_=w2c[s, g])
                    w2t.append(t)

                h = hp.tile([P, NCH, TOK], f8, tag="h", name=f"h_{s}")

                # ---- phase 1: h = silu(x@w1) * (x@w3), hi/lo split on chip ----
                for hg in PROC:
                    ps1 = psa.tile([P, TOK], f32, tag="ps", name=f"ps1_{s}_{hg}")
                    ps3 = psa.tile([P, TOK], f32, tag="ps", name=f"ps3_{s}_{hg}")
                    w = wt[hg]
                    for psx, khi, klo in ((ps1, 0, 1), (ps3, 2, 3)):
                        # x-lo correction covers K chunks 0..13 only (the
                        # last DoubleRow pair is skipped: spends ~1e-2 of the
                        # 2e-2 error budget for ~9us)
                        seq = ([(khi, xh, d) for d in range(KC // 2)]
                               + [(klo, xh, d) for d in range(KC // 2)]
                               + [(khi, xl, d) for d in range(KC // 2 - 1)])
                        for n, (wk, xx, d) in enumerate(seq):
                            nc.tensor.matmul(
                                out=psx[:],
                                lhsT=w[:, wk, 2 * d:2 * d + 2, :],
                                rhs=xx[:, 2 * d:2 * d + 2, :],
                                start=(n == 0),
                                stop=(n == len(seq) - 1),
                                perf_mode=DRM,
                            )
                    sil = tp.tile([P, TOK], f32, tag="sil", name=f"sil_{s}_{hg}")
                    nc.scalar.activation(sil[:], ps1[:], Act.Silu, scale=1.0 / SW1)
                    h16 = tp.tile([P, TOK], f32, tag="h16", name=f"h16_{s}_{hg}")
                    nc.vector.tensor_tensor(out=h16[:], in0=sil[:], in1=ps3[:],
                                            op=AluOpType.mult)
                    nc.gpsimd.tensor_copy(out=h[:, hg, :], in_=h16[:])
                    if hg == HC - 1:  # duplicate hh10 for the leftover pair
                        nc.gpsimd.tensor_copy(out=h[:, HC, :], in_=h16[:])
                    nc.vector.tensor_tensor(out=h[:, HC + 1 + hg, :], in0=h16[:],
                                            in1=h[:, hg, :], op=AluOpType.subtract)

                # ---- phase 2: out = (h_hi+h_lo) @ (w2_hi+w2_lo), 3 terms
                # covered by 17 DoubleRow pairs (see module docstring) ----
                HL = HC + 1  # h-lo chunk base (12)
                WL = HC + 1  # w2-lo chunk base (12); wl10 sits at 11
                # pair order: the 14 pairs whose h chunks are written by
                # mid-phase-1 come first; the 3 pairs touching the last
                # processed hid-chunk's writes (hh8/hh9 cast, hl8/hl9 sub)
                # come last, so phase 2 can start before phase 1's tail
                # elementwise ops land
                # the (hl10,Z) pair is dropped: hl10's correction is worth
                # ~4e-3 of error budget and a full DR per chain
                p2_pairs = (
                    [(2 * c, 2 * c) for c in range(4)]            # main c0-3
                    + [(2 * c, WL + 2 * c) for c in range(4)]     # w2-lo c0-3
                    + [(HL + 2 * c, 2 * c) for c in range(4)]     # h-lo c0-3
                    + [(HC - 1, HC - 1)]                          # hh10 x (wh10,wl10)
                    + [(8, 8), (8, WL + 8)]                       # main/w2-lo c4 (hh9)
                    + [(HL + 8, 8)]                               # h-lo c4 (hl9)
                )
                NEARLY = 13  # pairs with no dependency on the last hg's writes

                def p2_chain(ci, lo, hi, pso):
                    g, tk = ci // TK, ci % TK
                    for n in range(lo, hi):
                        hc, wc = p2_pairs[n]
                        nc.tensor.matmul(
                            out=pso[:],
                            lhsT=h[:, hc:hc + 2, tk * P:(tk + 1) * P],
                            rhs=w2t[g][:, wc:wc + 2, :],
                            start=(n == 0),
                            stop=(n == len(p2_pairs) - 1),
                            perf_mode=DRM,
                        )

                def p2_finish(ci, pso, o_sb):
                    g, tk = ci // TK, ci % TK
                    if g % 2 == 0:
                        o_sb[tk] = op.tile([P, 2 * NG], bf16, tag=f"o_{tk}",
                                           bufs=2, name=f"o_{s}_{g // 2}_{tk}")
                    o = o_sb[tk]
                    half = o[:, (g % 2) * NG:(g % 2 + 1) * NG]
                    if ci % 2 == 0:
                        nc.vector.tensor_scalar_mul(out=half, in0=pso[:],
                                                    scalar1=OSC)
                    else:
                        nc.scalar.activation(half, pso[:], Act.Copy, scale=OSC)
                    if g % 2 == 1:
                        nc.gpsimd.dma_start(
                            out=out[s * TOK + tk * P:s * TOK + (tk + 1) * P,
                                    (g // 2) * 2 * NG:(g // 2 + 1) * 2 * NG],
                            in_=o[:],
                        )

                o_sb = {}
                NP2 = len(p2_pairs)
                # first three chains interleaved: their early pairs run while
                # the last hg's h writes land, then their tails complete
                first = [psb.tile([P, NG], f32, tag="ps", name=f"pso_{s}_{ci}")
                         for ci in range(3)]
                for ci in range(3):
                    p2_chain(ci, 0, NEARLY, first[ci])
                for ci in range(3):
                    p2_chain(ci, NEARLY, NP2, first[ci])
                    p2_finish(ci, first[ci], o_sb)
                for ci in range(3, NGRP * TK):
                    pso = psb.tile([P, NG], f32, tag="ps", name=f"pso_{s}_{ci}")
                    p2_chain(ci, 0, NP2, pso)
                    p2_finish(ci, pso, o_sb)
    nc.compile()
    return nc


def _plan(m_sizes, T):
    """Mirror the reference routing: contiguous segments by expert, then chop
    into TOK-sized chunks and deal them contiguously across cores."""
    bounds = np.cumsum(np.asarray(m_sizes, dtype=np.int64))
    E = len(bounds)
    chunks = []  # (expert, row_start, nrows)
    prev = 0
    for e in range(E):
        lo, hi = prev, min(int(bounds[e]), T)
        prev = max(lo, hi)
        seg = hi - lo
        off = lo
        while seg > 0:
            take = min(TOK, seg)
            chunks.append((e, off, take))
            off += take
            seg -= take
    S = max(1, math.ceil(len(chunks) / N_CORES))
    while len(chunks) < N_CORES * S:
        chunks.append((0, 0, 0))  # dummy slot
    per_core = [chunks[c * S:(c + 1) * S] for c in range(N_CORES)]
    return per_core, S


def _hilo(a):
    hi = a.astype(F8)
    lo = (a - hi.astype(np.float32)).astype(F8)
    return hi, lo


def _quant_weights(w1, w2, w3):
    """Per-expert hi/lo fp8 weights in the on-device layouts."""
    E = w1.shape[0]
    w13_e = np.empty((E, HC, P, 4, KC, P), dtype=F8)
    w2_e = np.empty((E, NGRP, P, NCW, NG), dtype=F8)

    def t13(a):  # [D, H] -> [HC, P(k), KC, P(h)]
        return a.reshape(KC, P, HC, P).transpose(2, 1, 0, 3)

    def t2(a):  # [H, D] -> [NGRP, P(h), HC, NG]
        return a.reshape(HC, P, NGRP, NG).transpose(2, 1, 0, 3)

    for e in range(E):
        h1, l1 = _hilo(w1[e] * SW1)
        h3, l3 = _hilo(w3[e] * SW3)
        w13_e[e, :, :, 0] = t13(h1)
        w13_e[e, :, :, 1] = t13(l1)
        w13_e[e, :, :, 2] = t13(h3)
        w13_e[e, :, :, 3] = t13(l3)
        h2, l2 = _hilo(w2[e] * SW2)
        th, tl = t2(h2), t2(l2)
        w2_e[e, :, :, 0:HC] = th          # wh0..wh10
        w2_e[e, :, :, HC] = tl[:, :, HC - 1]   # wl10
        w2_e[e, :, :, HC + 1:NCW] = tl[:, :, 0:HC - 1]  # wl0..wl9
    return w13_e, w2_e


def kernel(x, w1, w2, w3, m_sizes, _trace=False):
    x = np.asarray(x, dtype=np.float32)
    w1 = np.asarray(w1, dtype=np.float32)
    w2 = np.asarray(w2, dtype=np.float32)
    w3 = np.asarray(w3, dtype=np.float32)
    T = x.shape[0]
    assert x.shape[1] == D and w1.shape[1:] == (D, H), (x.shape, w1.shape)
    assert w2.shape[1:] == (H, D) and w3.shape[1:] == (D, H), (w2.shape, w3.shape)

    per_core, S = _plan(m_sizes, T)

    if S not in _compiled_cache:
        _compiled_cache[S] = _build_program(S)
    nc = _compiled_cache[S]

    wkey = (id(w1), id(w2), id(w3))
    if wkey not in _wq_cache:
        _wq_cache.clear()
        _wq_cache[wkey] = _quant_weights(w1, w2, w3)
    w13_e, w2_e = _wq_cache[wkey]

    in_maps = []
    for c in range(N_CORES):
        slots = per_core[c]
        xt_c = np.zeros((S, 2, P, KC, TOK), dtype=F8)
        for s, (e, off, ln) in enumerate(slots):
            if ln:
                seg = np.zeros((TOK, D), dtype=np.float32)
                seg[:ln] = x[off:off + ln]
                sh, sl = _hilo(seg)
                # [TOK, D] -> [P(k), KC, TOK]
                xt_c[s, 0] = sh.reshape(TOK, KC, P).transpose(2, 1, 0)
                xt_c[s, 1] = sl.reshape(TOK, KC, P).transpose(2, 1, 0)
        eids = [e for (e, _, _) in slots]
        in_maps.append({
            "xt": xt_c,
            "w13": np.ascontiguousarray(w13_e[eids]),
            "w2c": np.ascontiguousarray(w2_e[eids]),
        })

    try:
        res = run_bass_kernel_spmd(
            nc, in_maps, list(range(N_CORES)), trace=_trace,
        )
    except Exception:
        # transient NRT device errors have been observed once after a fresh
        # compile; a single retry is free if the device truly died
        res = run_bass_kernel_spmd(
            nc, in_maps, list(range(N_CORES)), trace=_trace,
        )

    full = np.zeros((T, D), dtype=np.float32)
    for c in range(N_CORES):
        oc = res.results[c]["out"].astype(np.float32)
        for s, (e, off, ln) in enumerate(per_core[c]):
            if ln:
                full[off:off + ln] = oc[s * TOK:s * TOK + ln]

    last_run_info.clear()
    last_run_info.update({
        "exec_time_ns": res.exec_time_ns,
        "profile_json": getattr(res, "profile_json", None),
        "S": S,
    })
    return full


# revision 35
# speedup vs baseline: 1.3812x; 1.0001x over previous
"""Grouped SwiGLU experts (MoE, contiguous per-expert token segments) on 8 trn2 cores.

Strategy: expert-parallel over 512-token slots (as the bf16 baseline), but all
matmuls run in fp8-e4m3 with MatmulPerfMode.DoubleRow (K=256 per instruction,
0.5 cycles/row -> 4x bf16 FLOP rate).  Plain fp8 is ~6.5% rel err, far over
the 2e-2 budget, so every tensor is carried as an exact-ish hi+lo e4m3 pair
and each GEMM computes the three significant cross terms
    a@b ~= a_hi@b_hi + a_hi@b_lo + a_lo@b_hi        (lo@lo ~ 1e-3 rel, dropped)
The lo parts keep the SAME scale as their hi parts, so all three terms
accumulate into a single PSUM bank with no fixup arithmetic (end-to-end rel
err ~3e-3 measured in numpy).  Net tensor-engine time is 0.75x rows vs 1.0x
for bf16 -> ~1.3x speedup.

Scales keep everything in e4m3's happy range: x at 1, w1 at 64, w3 at 16,
w2 at 64; h is produced at scale 16 (= silu(ps1/64) * ps3), output PSUM is
scale 1024 and is written back as bf16 after a 1/1024 scale.

H=1408 is 11 128-chunks (odd = 33 product chunks over the 3 terms), so the
phase-2 chunk layouts are arranged to cover all 33 in exactly 17 DoubleRow
pairs: h is [hh0..hh10, hh10(dup), hl0..hl10, Z] and w2 is
[wh0..wh10, wl10, wl0..wl9].  Pairs:
  5x (hh2c,hh2c+1)x(wh2c,wh2c+1)   main
  1x (hh10,hh10')x(wh10,wl10)      both hh10 leftovers in one pair
  5x (hh2c,hh2c+1)x(wl2c,wl2c+1)   w2-lo correction
  5x (hl2c,hl2c+1)x(wh2c,wh2c+1)   h-lo correction
  1x (hl10,Z)x(wh10,wl10)          last h-lo leftover

kernel(**inputs) -> full [16384, 2048] fp32 output.  Self-contained.
"""

import math

import numpy as np
import ml_dtypes

import concourse.bass as bass
import concourse.tile as tile
from concourse import bacc
from concourse import mybir
from concourse.alu_op_type import AluOpType
from concourse.bass_utils import run_bass_kernel_spmd

N_CORES = 8
D = 2048          # dim_in
H = 1408          # dim_hidden
TOK = 512         # tokens per slot
P = 128           # partitions
KC = D // P       # 16 k-chunks over dim_in
HC = H // P       # 11 hid-chunks
TK = TOK // P     # 4 token tiles per slot
NG = 512          # phase-2 out-column group width
NGRP = D // NG    # 4 column groups
NCH = 24          # h chunk axis: [hh*11, hh10dup, hl*11, Z]
NCW = 22          # w2 chunk axis: [wh*11, wl10, wl0..wl9]
WR = 5            # w13 tag rotation depth (SBUF vs prefetch tradeoff)

SW1 = 64.0        # w1 quant scale
SW3 = 16.0        # w3 quant scale -> h comes out at scale 16
SW2 = 64.0        # w2 quant scale
OSC = 1.0 / (16.0 * 64.0)  # final psum -> out scale

F8 = ml_dtypes.float8_e4m3

_compiled_cache = {}
_wq_cache = {}
last_run_info = {}


def _build_program(S):
    """Per-core SPMD program: S slots, each 512 tokens of one expert."""
    nc = bacc.Bacc()
    f8 = mybir.dt.float8e4
    f32 = mybir.dt.float32
    bf16 = mybir.dt.bfloat16
    DRM = mybir.MatmulPerfMode.DoubleRow
    Act = mybir.ActivationFunctionType

    xt = nc.declare_dram_parameter("xt", [S, 2, P, KC, TOK], f8, isOutput=False)
    w13 = nc.declare_dram_parameter("w13", [S, HC, P, 4, KC, P], f8, isOutput=False)
    w2c = nc.declare_dram_parameter("w2c", [S, NGRP, P, NCW, NG], f8, isOutput=False)
    out = nc.declare_dram_parameter("out", [S * TOK, D], bf16, isOutput=True)

    with tile.TileContext(nc) as tc:
        with (
            tc.tile_pool(name="xp", bufs=2) as xp,
            tc.tile_pool(name="wp", bufs=1) as wp,
            tc.tile_pool(name="w2p", bufs=1) as w2p,
            tc.tile_pool(name="hp", bufs=2) as hp,
            tc.tile_pool(name="tp", bufs=2) as tp,
            tc.tile_pool(name="op", bufs=4) as op,
            tc.tile_pool(name="psA", bufs=5, space="PSUM") as psa,
            tc.tile_pool(name="psB", bufs=3, space="PSUM") as psb,
        ):
            # phase-1 processing order: hg10 runs mid-phase so its h chunks
            # (hh10, the dup, hl10) are long done before phase 2 reads them
            # in its final DoubleRow pairs
            PROC = [0, 1, 2, 3, 4, 10, 5, 6, 7, 8, 9]

            # PE pstate warmup: ~150 tiny self-contained matmuls on scratch
            # data keep the PE busy through the initial DMA wait so the
            # first real chains run at full clock
            scr_w = tp.tile([P, 2, P], f8, tag="scrw", bufs=1, name="scr_w")
            scr_x = tp.tile([P, 2, 64], f8, tag="scrx", bufs=1, name="scr_x")
            nc.gpsimd.memset(scr_w[:], 0)
            nc.gpsimd.memset(scr_x[:], 0)
            scr_ps = psa.tile([P, TOK], f32, tag="ps", name="scr_ps")
            for i in range(150):
                nc.tensor.matmul(out=scr_ps[:, 0:64], lhsT=scr_w[:], rhs=scr_x[:],
                                 start=True, stop=True, perf_mode=DRM)

            for s in range(S):
                # ---- loads.  All on the SP queue, which is a strict
                # blocking FIFO (a DMA holds the SEQ during its semaphore
                # waits), so issue order IS priority order.  w13 issues are
                # paced by their tag-rotation frees; by the time the FIFO
                # reaches this slot's w2 issues, the previous slot's phase 2
                # is done, so they never block later loads. ----
                xh = xp.tile([P, KC, TOK], f8, tag="xh", name=f"xh_{s}")
                xl = xp.tile([P, KC, TOK], f8, tag="xl", name=f"xl_{s}")
                wt = {}

                def load_w13(pos, s=s, wt=wt):
                    hg = PROC[pos]
                    t = wp.tile([P, 4, KC, P], f8, tag=f"w_{pos % WR}",
                                name=f"w13_{s}_{hg}")
                    nc.sync.dma_start(out=t[:], in_=w13[s, hg])
                    wt[hg] = t

                KH = KC // 2
                if s == 0:
                    # fine-grained first loads: the first chain can start
                    # after one x quarter + half a w13 kind instead of 2MB
                    t = wp.tile([P, 4, KC, P], f8, tag="w_0", name="w13_0_0")
                    wt[0] = t
                    nc.sync.dma_start(out=t[:, 0, 0:KH, :], in_=w13[0, 0, :, 0, 0:KH])
                    nc.sync.dma_start(out=xh[:, 0:4, :], in_=xt[0, 0, :, 0:4])
                    nc.sync.dma_start(out=t[:, 0, KH:KC, :], in_=w13[0, 0, :, 0, KH:KC])
                    nc.sync.dma_start(out=xh[:, 4:8, :], in_=xt[0, 0, :, 4:8])
                    nc.sync.dma_start(out=t[:, 1], in_=w13[0, 0, :, 1])
                    nc.sync.dma_start(out=xh[:, 8:12, :], in_=xt[0, 0, :, 8:12])
                    nc.sync.dma_start(out=xh[:, 12:16, :], in_=xt[0, 0, :, 12:16])
                    nc.sync.dma_start(out=t[:, 2], in_=w13[0, 0, :, 2])
                    nc.sync.dma_start(out=t[:, 3], in_=w13[0, 0, :, 3])
                    nc.sync.dma_start(out=xl[:, 0:KH, :], in_=xt[0, 1, :, 0:KH])
                    nc.sync.dma_start(out=xl[:, KH:KC, :], in_=xt[0, 1, :, KH:KC])
                else:
                    nc.sync.dma_start(out=xh[:, 0:KH, :], in_=xt[s, 0, :, 0:KH])
                    load_w13(0)
                    nc.sync.dma_start(out=xh[:, KH:KC, :], in_=xt[s, 0, :, KH:KC])
                    nc.sync.dma_start(out=xl[:, 0:KH, :], in_=xt[s, 1, :, 0:KH])
                    nc.sync.dma_start(out=xl[:, KH:KC, :], in_=xt[s, 1, :, KH:KC])
                for pos in range(1, HC):
                    load_w13(pos)
                w2t = []
                for g in range(NGRP):
                    t = w2p.tile([P, NCW, NG], f8, tag=f"w2_{g}",
                                 name=f"w2_{s}_{g}")
                    nc.sync.dma_start(out=t[:], in_=w2c[s, g])
                    w2t.append(t)

                h = hp.tile([P, NCH, TOK], f8, tag="h", name=f"h_{s}")

                # ---- phase 1: h = silu(x@w1) * (x@w3), hi/lo split on chip ----
                for pi, hg in enumerate(PROC):
                    ps1 = psa.tile([P, TOK], f32, tag="ps", name=f"ps1_{s}_{hg}")
                    ps3 = psa.tile([P, TOK], f32, tag="ps", name=f"ps3_{s}_{hg}")
                    w = wt[hg]

                    # x-lo correction covers K chunks 0..13 only (the last
                    # DoubleRow pair is skipped: spends ~1e-2 of the 2e-2
                    # error budget for ~9us)
                    def chain(psx, khi, klo, part):
                        seq = ([(khi, xh, d, n == 0, False)
                                for n, d in enumerate(range(KC // 2))]
                               + [(klo, xh, d, False, False)
                                  for d in range(KC // 2)]
                               + [(khi, xl, d, False, d == KC // 2 - 2)
                                  for d in range(KC // 2 - 1)])
                        lo, hi = (0, 2 * (KC // 2)) if part == 0 else \
                                 (2 * (KC // 2), len(seq)) if part == 1 else \
                                 (0, len(seq))
                        for wk, xx, d, st, sp in seq[lo:hi]:
                            nc.tensor.matmul(
                                out=psx[:], lhsT=w[:, wk, 2 * d:2 * d + 2, :],
                                rhs=xx[:, 2 * d:2 * d + 2, :],
                                start=st, stop=sp, perf_mode=DRM,
                            )

                    if pi < 2:
                        # head of the slot: both hi-term chains first (their
                        # weight tiles arrive before x-lo does on slot 0)
                        chain(ps1, 0, 1, 0)
                        chain(ps3, 2, 3, 0)
                        chain(ps1, 0, 1, 1)
                        chain(ps3, 2, 3, 1)
                    else:
                        chain(ps1, 0, 1, 2)
                        chain(ps3, 2, 3, 2)
                    sil = tp.tile([P, TOK], f32, tag="sil", name=f"sil_{s}_{hg}")
                    nc.scalar.activation(sil[:], ps1[:], Act.Silu, scale=1.0 / SW1)
                    h16 = tp.tile([P, TOK], f32, tag="h16", name=f"h16_{s}_{hg}")
                    nc.vector.tensor_tensor(out=h16[:], in0=sil[:], in1=ps3[:],
                                            op=AluOpType.mult)
                    nc.gpsimd.tensor_copy(out=h[:, hg, :], in_=h16[:])
                    if hg == HC - 1:  # duplicate hh10 for the leftover pair
                        nc.gpsimd.tensor_copy(out=h[:, HC, :], in_=h16[:])
                    nc.vector.tensor_tensor(out=h[:, HC + 1 + hg, :], in0=h16[:],
                                            in1=h[:, hg, :], op=AluOpType.subtract)

                # ---- phase 2: out = (h_hi+h_lo) @ (w2_hi+w2_lo), 3 terms
                # covered by 17 DoubleRow pairs (see module docstring) ----
                HL = HC + 1  # h-lo chunk base (12)
                WL = HC + 1  # w2-lo chunk base (12); wl10 sits at 11
                # pair order: the 14 pairs whose h chunks are written by
                # mid-phase-1 come first; the 3 pairs touching the last
                # processed hid-chunk's writes (hh8/hh9 cast, hl8/hl9 sub)
                # come last, so phase 2 can start before phase 1's tail
                # elementwise ops land
                # the (hl10,Z) pair is dropped: hl10's correction is worth
                # ~4e-3 of error budget and a full DR per chain
                p2_pairs = (
                    [(2 * c, 2 * c) for c in range(4)]            # main c0-3
                    + [(2 * c, WL + 2 * c) for c in range(4)]     # w2-lo c0-3
                    + [(HL + 2 * c, 2 * c) for c in range(4)]     # h-lo c0-3
                    + [(HC - 1, HC - 1)]                          # hh10 x (wh10,wl10)
                    + [(8, 8), (8, WL + 8)]                       # main/w2-lo c4 (hh9)
                    + [(HL + 8, 8)]                               # h-lo c4 (hl9)
                )
                NEARLY = 13  # pairs with no dependency on the last hg's writes

                def p2_chain(ci, lo, hi, pso):
                    g, tk = ci // TK, ci % TK
                    for n in range(lo, hi):
                        hc, wc = p2_pairs[n]
                        nc.tensor.matmul(
                            out=pso[:],
                            lhsT=h[:, hc:hc + 2, tk * P:(tk + 1) * P],
                            rhs=w2t[g][:, wc:wc + 2, :],
                            start=(n == 0),
                            stop=(n == len(p2_pairs) - 1),
                            perf_mode=DRM,
                        )

                def p2_finish(ci, pso, o_sb):
                    g, tk = ci // TK, ci % TK
                    last = (ci == NGRP * TK - 1)
                    if g % 2 == 0:
                        o_sb[tk] = op.tile([P, 2 * NG], bf16, tag=f"o_{tk}",
                                           bufs=2, name=f"o_{s}_{g // 2}_{tk}")
                    o = o_sb[tk]
                    half = o[:, (g % 2) * NG:(g % 2 + 1) * NG]
                    rows = slice(s * TOK + tk * P, s * TOK + (tk + 1) * P)
                    if last:
                        # final chain of the slot: split the copy across two
                        # engines and store its half separately so the slot
                        # tail is as short as possible
                        nc.vector.tensor_scalar_mul(out=o[:, NG:NG + NG // 2],
                                                    in0=pso[:, 0:NG // 2],
                                                    scalar1=OSC)
                        nc.scalar.activation(o[:, NG + NG // 2:2 * NG],
                                             pso[:, NG // 2:NG], Act.Copy,
                                             scale=OSC)
                        nc.gpsimd.dma_start(
                            out=out[rows, (2 * (g // 2) + 1) * NG:
                                    (2 * (g // 2) + 2) * NG],
                            in_=o[:, NG:2 * NG],
                        )
                        return
                    if ci % 2 == 0:
                        nc.vector.tensor_scalar_mul(out=half, in0=pso[:],
                                                    scalar1=OSC)
                    else:
                        nc.scalar.activation(half, pso[:], Act.Copy, scale=OSC)
                    if g % 2 == 1:
                        nc.gpsimd.dma_start(
                            out=out[rows,
                                    (g // 2) * 2 * NG:(g // 2 + 1) * 2 * NG],
                            in_=o[:],
                        )
                    elif g == NGRP - 2 and tk == TK - 1:
                        # the last chain (g3,tk3) stores its own half; ship
                        # this g2 half now instead of pairing with it
                        nc.gpsimd.dma_start(
                            out=out[rows, (g // 2) * 2 * NG:
                                    (g // 2) * 2 * NG + NG],
                            in_=o[:, 0:NG],
                        )

                o_sb = {}
                NP2 = len(p2_pairs)
                # first three chains interleaved: their early pairs run while
                # the last hg's h writes land, then their tails complete
                first = [psb.tile([P, NG], f32, tag="ps", name=f"pso_{s}_{ci}")
                         for ci in range(3)]
                for ci in range(3):
                    p2_chain(ci, 0, NEARLY, first[ci])
                for ci in range(3):
                    p2_chain(ci, NEARLY, NP2, first[ci])
                    p2_finish(ci, first[ci], o_sb)
                for ci in range(3, NGRP * TK):
                    pso = psb.tile([P, NG], f32, tag="ps", name=f"pso_{s}_{ci}")
                    p2_chain(ci, 0, NP2, pso)
                    p2_finish(ci, pso, o_sb)
    nc.compile()
    return nc


def _plan(m_sizes, T):
    """Mirror the reference routing: contiguous segments by expert, then chop
    into TOK-sized chunks and deal them contiguously across cores."""
    bounds = np.cumsum(np.asarray(m_sizes, dtype=np.int64))
    E = len(bounds)
    chunks = []  # (expert, row_start, nrows)
    prev = 0
    for e in range(E):
        lo, hi = prev, min(int(bounds[e]), T)
        prev = max(lo, hi)
        seg = hi - lo
        off = lo
        while seg > 0:
            take = min(TOK, seg)
            chunks.append((e, off, take))
            off += take
            seg -= take
    S = max(1, math.ceil(len(chunks) / N_CORES))
    while len(chunks) < N_CORES * S:
        chunks.append((0, 0, 0))  # dummy slot
    per_core = [chunks[c * S:(c + 1) * S] for c in range(N_CORES)]
    return per_core, S


def _hilo(a):
    hi = a.astype(F8)
    lo = (a - hi.astype(np.float32)).astype(F8)
    return hi, lo


def _quant_weights(w1, w2, w3):
    """Per-expert hi/lo fp8 weights in the on-device layouts."""
    E = w1.shape[0]
    w13_e = np.empty((E, HC, P, 4, KC, P), dtype=F8)
    w2_e = np.empty((E, NGRP, P, NCW, NG), dtype=F8)

    def t13(a):  # [D, H] -> [HC, P(k), KC, P(h)]
        return a.reshape(KC, P, HC, P).transpose(2, 1, 0, 3)

    def t2(a):  # [H, D] -> [NGRP, P(h), HC, NG]
        return a.reshape(HC, P, NGRP, NG).transpose(2, 1, 0, 3)

    for e in range(E):
        h1, l1 = _hilo(w1[e] * SW1)
        h3, l3 = _hilo(w3[e] * SW3)
        w13_e[e, :, :, 0] = t13(h1)
        w13_e[e, :, :, 1] = t13(l1)
        w13_e[e, :, :, 2] = t13(h3)
        w13_e[e, :, :, 3] = t13(l3)
        h2, l2 = _hilo(w2[e] * SW2)
        th, tl = t2(h2), t2(l2)
        w2_e[e, :, :, 0:HC] = th          # wh0..wh10
        w2_e[e, :, :, HC] = tl[:, :, HC - 1]   # wl10
        w2_e[e, :, :, HC + 1:NCW] = tl[:, :, 0:HC - 1]  # wl0..wl9
    return w13_e, w2_e


def kernel(x, w1, w2, w3, m_sizes, _trace=False):
    x = np.asarray(x, dtype=np.float32)
    w1 = np.asarray(w1, dtype=np.float32)
    w2 = np.asarray(w2, dtype=np.float32)
    w3 = np.asarray(w3, dtype=np.float32)
    T = x.shape[0]
    assert x.shape[1] == D and w1.shape[1:] == (D, H), (x.shape, w1.shape)
    assert w2.shape[1:] == (H, D) and w3.shape[1:] == (D, H), (w2.shape, w3.shape)

    per_core, S = _plan(m_sizes, T)

    if S not in _compiled_cache:
        _compiled_cache[S] = _build_program(S)
    nc = _compiled_cache[S]

    wkey = (id(w1), id(w2), id(w3))
    if wkey not in _wq_cache:
        _wq_cache.clear()
        _wq_cache[wkey] = _quant_weights(w1, w2, w3)
    w13_e, w2_e = _wq_cache[wkey]

    in_maps = []
    for c in range(N_CORES):
        slots = per_core[c]
        xt_c = np.zeros((S, 2, P, KC, TOK), dtype=F8)
        for s, (e, off, ln) in enumerate(slots):
            if ln:
                seg = np.zeros((TOK, D), dtype=np.float32)
                seg[:ln] = x[off:off + ln]
                sh, sl = _hilo(seg)
                # [TOK, D] -> [P(k), KC, TOK]
                xt_c[s, 0] = sh.reshape(TOK, KC, P).transpose(2, 1, 0)
                xt_c[s, 1] = sl.reshape(TOK, KC, P).transpose(2, 1, 0)
        eids = [e for (e, _, _) in slots]
        in_maps.append({
            "xt": xt_c,
            "w13": np.ascontiguousarray(w13_e[eids]),
            "w2c": np.ascontiguousarray(w2_e[eids]),
        })

    try:
        res = run_bass_kernel_spmd(
            nc, in_maps, list(range(N_CORES)), trace=_trace,
        )
    except Exception:
        # transient NRT device errors have been observed once after a fresh
        # compile; a single retry is free if the device truly died
        res = run_bass_kernel_spmd(
            nc, in_maps, list(range(N_CORES)), trace=_trace,
        )

    full = np.zeros((T, D), dtype=np.float32)
    for c in range(N_CORES):
        oc = res.results[c]["out"].astype(np.float32)
        for s, (e, off, ln) in enumerate(per_core[c]):
            if ln:
                full[off:off + ln] = oc[s * TOK:s * TOK + ln]

    last_run_info.clear()
    last_run_info.update({
        "exec_time_ns": res.exec_time_ns,
        "profile_json": getattr(res, "profile_json", None),
        "S": S,
    })
    return full


# revision 37
# speedup vs baseline: 1.3827x; 1.0011x over previous
"""Grouped SwiGLU experts (MoE, contiguous per-expert token segments) on 8 trn2 cores.

Strategy: expert-parallel over 512-token slots (as the bf16 baseline), but all
matmuls run in fp8-e4m3 with MatmulPerfMode.DoubleRow (K=256 per instruction,
0.5 cycles/row -> 4x bf16 FLOP rate).  Plain fp8 is ~6.5% rel err, far over
the 2e-2 budget, so every tensor is carried as an exact-ish hi+lo e4m3 pair
and each GEMM computes the three significant cross terms
    a@b ~= a_hi@b_hi + a_hi@b_lo + a_lo@b_hi        (lo@lo ~ 1e-3 rel, dropped)
The lo parts keep the SAME scale as their hi parts, so all three terms
accumulate into a single PSUM bank with no fixup arithmetic (end-to-end rel
err ~3e-3 measured in numpy).  Net tensor-engine time is 0.75x rows vs 1.0x
for bf16 -> ~1.3x speedup.

Scales keep everything in e4m3's happy range: x at 1, w1 at 64, w3 at 16,
w2 at 64; h is produced at scale 16 (= silu(ps1/64) * ps3), output PSUM is
scale 1024 and is written back as bf16 after a 1/1024 scale.

H=1408 is 11 128-chunks (odd = 33 product chunks over the 3 terms), so the
phase-2 chunk layouts are arranged to cover all 33 in exactly 17 DoubleRow
pairs: h is [hh0..hh10, hh10(dup), hl0..hl10, Z] and w2 is
[wh0..wh10, wl10, wl0..wl9].  Pairs:
  5x (hh2c,hh2c+1)x(wh2c,wh2c+1)   main
  1x (hh10,hh10')x(wh10,wl10)      both hh10 leftovers in one pair
  5x (hh2c,hh2c+1)x(wl2c,wl2c+1)   w2-lo correction
  5x (hl2c,hl2c+1)x(wh2c,wh2c+1)   h-lo correction
  1x (hl10,Z)x(wh10,wl10)          last h-lo leftover

kernel(**inputs) -> full [16384, 2048] fp32 output.  Self-contained.
"""

import math

import numpy as np
import ml_dtypes

import concourse.bass as bass
import concourse.tile as tile
from concourse import bacc
from concourse import mybir
from concourse.alu_op_type import AluOpType
from concourse.bass_utils import run_bass_kernel_spmd

N_CORES = 8
D = 2048          # dim_in
H = 1408          # dim_hidden
TOK = 512         # tokens per slot
P = 128           # partitions
KC = D // P       # 16 k-chunks over dim_in
HC = H // P       # 11 hid-chunks
TK = TOK // P     # 4 token tiles per slot
NG = 512          # phase-2 out-column group width
NGRP = D // NG    # 4 column groups
NCH = 24          # h chunk axis: [hh*11, hh10dup, hl*11, Z]
NCW = 22          # w2 chunk axis: [wh*11, wl10, wl0..wl9]
WR = 5            # w13 tag rotation depth (SBUF vs prefetch tradeoff)

SW1 = 64.0        # w1 quant scale
SW3 = 16.0        # w3 quant scale -> h comes out at scale 16
SW2 = 64.0        # w2 quant scale
OSC = 1.0 / (16.0 * 64.0)  # final psum -> out scale

F8 = ml_dtypes.float8_e4m3

_compiled_cache = {}
_wq_cache = {}
last_run_info = {}


def _build_program(S):
    """Per-core SPMD program: S slots, each 512 tokens of one expert."""
    nc = bacc.Bacc()
    f8 = mybir.dt.float8e4
    f32 = mybir.dt.float32
    bf16 = mybir.dt.bfloat16
    DRM = mybir.MatmulPerfMode.DoubleRow
    Act = mybir.ActivationFunctionType

    xt = nc.declare_dram_parameter("xt", [S, 2, P, KC, TOK], f8, isOutput=False)
    w13 = nc.declare_dram_parameter("w13", [S, HC, P, 4, KC, P], f8, isOutput=False)
    w2c = nc.declare_dram_parameter("w2c", [S, NGRP, P, NCW, NG], f8, isOutput=False)
    out = nc.declare_dram_parameter("out", [S * TOK, D], bf16, isOutput=True)

    with tile.TileContext(nc) as tc:
        with (
            tc.tile_pool(name="xp", bufs=2) as xp,
            tc.tile_pool(name="wp", bufs=1) as wp,
            tc.tile_pool(name="w2p", bufs=1) as w2p,
            tc.tile_pool(name="hp", bufs=2) as hp,
            tc.tile_pool(name="tp", bufs=2) as tp,
            tc.tile_pool(name="op", bufs=4) as op,
            tc.tile_pool(name="psA", bufs=5, space="PSUM") as psa,
            tc.tile_pool(name="psB", bufs=3, space="PSUM") as psb,
        ):
            # phase-1 processing order: hg10 runs mid-phase so its h chunks
            # (hh10, the dup, hl10) are long done before phase 2 reads them
            # in its final DoubleRow pairs
            PROC = [0, 1, 2, 3, 4, 10, 5, 6, 7, 8, 9]

            # PE pstate warmup: ~150 tiny self-contained matmuls on scratch
            # data keep the PE busy through the initial DMA wait so the
            # first real chains run at full clock
            scr_w = tp.tile([P, 2, P], f8, tag="scrw", bufs=1, name="scr_w")
            scr_x = tp.tile([P, 2, 64], f8, tag="scrx", bufs=1, name="scr_x")
            nc.gpsimd.memset(scr_w[:], 0)
            nc.gpsimd.memset(scr_x[:], 0)
            scr_ps = psa.tile([P, TOK], f32, tag="ps", name="scr_ps")
            for i in range(150):
                nc.tensor.matmul(out=scr_ps[:, 0:64], lhsT=scr_w[:], rhs=scr_x[:],
                                 start=True, stop=True, perf_mode=DRM)

            for s in range(S):
                # ---- loads.  All on the SP queue, which is a strict
                # blocking FIFO (a DMA holds the SEQ during its semaphore
                # waits), so issue order IS priority order.  w13 issues are
                # paced by their tag-rotation frees; by the time the FIFO
                # reaches this slot's w2 issues, the previous slot's phase 2
                # is done, so they never block later loads. ----
                xh = xp.tile([P, KC, TOK], f8, tag="xh", name=f"xh_{s}")
                xl = xp.tile([P, KC, TOK], f8, tag="xl", name=f"xl_{s}")
                wt = {}

                def load_w13(pos, s=s, wt=wt):
                    hg = PROC[pos]
                    t = wp.tile([P, 4, KC, P], f8, tag=f"w_{pos % WR}",
                                name=f"w13_{s}_{hg}")
                    nc.sync.dma_start(out=t[:], in_=w13[s, hg])
                    wt[hg] = t

                KH = KC // 2
                if s == 0:
                    # fine-grained first loads: the first chain can start
                    # after one x quarter + half a w13 kind instead of 2MB
                    t = wp.tile([P, 4, KC, P], f8, tag="w_0", name="w13_0_0")
                    wt[0] = t
                    nc.sync.dma_start(out=t[:, 0, 0:KH, :], in_=w13[0, 0, :, 0, 0:KH])
                    nc.sync.dma_start(out=xh[:, 0:4, :], in_=xt[0, 0, :, 0:4])
                    nc.sync.dma_start(out=t[:, 0, KH:KC, :], in_=w13[0, 0, :, 0, KH:KC])
                    nc.sync.dma_start(out=xh[:, 4:8, :], in_=xt[0, 0, :, 4:8])
                    nc.sync.dma_start(out=t[:, 1], in_=w13[0, 0, :, 1])
                    nc.sync.dma_start(out=xh[:, 8:12, :], in_=xt[0, 0, :, 8:12])
                    nc.sync.dma_start(out=xh[:, 12:16, :], in_=xt[0, 0, :, 12:16])
                    nc.sync.dma_start(out=t[:, 2], in_=w13[0, 0, :, 2])
                    nc.sync.dma_start(out=t[:, 3], in_=w13[0, 0, :, 3])
                    # x-lo chunks 14,15 are never used (skipped correction)
                    nc.sync.dma_start(out=xl[:, 0:KH, :], in_=xt[0, 1, :, 0:KH])
                    nc.sync.dma_start(out=xl[:, KH:KC - 2, :],
                                      in_=xt[0, 1, :, KH:KC - 2])
                else:
                    nc.sync.dma_start(out=xh[:, 0:KH, :], in_=xt[s, 0, :, 0:KH])
                    load_w13(0)
                    nc.sync.dma_start(out=xh[:, KH:KC, :], in_=xt[s, 0, :, KH:KC])
                    nc.sync.dma_start(out=xl[:, 0:KH, :], in_=xt[s, 1, :, 0:KH])
                    nc.sync.dma_start(out=xl[:, KH:KC - 2, :],
                                      in_=xt[s, 1, :, KH:KC - 2])
                for pos in range(1, HC):
                    load_w13(pos)
                w2t = []
                for g in range(NGRP):
                    t = w2p.tile([P, NCW, NG], f8, tag=f"w2_{g}",
                                 name=f"w2_{s}_{g}")
                    nc.sync.dma_start(out=t[:], in_=w2c[s, g])
                    w2t.append(t)

                h = hp.tile([P, NCH, TOK], f8, tag="h", name=f"h_{s}")

                # ---- phase 1: h = silu(x@w1) * (x@w3), hi/lo split on chip ----
                for pi, hg in enumerate(PROC):
                    ps1 = psa.tile([P, TOK], f32, tag="ps", name=f"ps1_{s}_{hg}")
                    ps3 = psa.tile([P, TOK], f32, tag="ps", name=f"ps3_{s}_{hg}")
                    w = wt[hg]

                    # x-lo correction covers K chunks 0..13 only (the last
                    # DoubleRow pair is skipped: spends ~1e-2 of the 2e-2
                    # error budget for ~9us)
                    def chain(psx, khi, klo, part):
                        seq = ([(khi, xh, d, n == 0, False)
                                for n, d in enumerate(range(KC // 2))]
                               + [(klo, xh, d, False, False)
                                  for d in range(KC // 2)]
                               + [(khi, xl, d, False, d == KC // 2 - 2)
                                  for d in range(KC // 2 - 1)])
                        lo, hi = (0, 2 * (KC // 2)) if part == 0 else \
                                 (2 * (KC // 2), len(seq)) if part == 1 else \
                                 (0, len(seq))
                        for wk, xx, d, st, sp in seq[lo:hi]:
                            nc.tensor.matmul(
                                out=psx[:], lhsT=w[:, wk, 2 * d:2 * d + 2, :],
                                rhs=xx[:, 2 * d:2 * d + 2, :],
                                start=st, stop=sp, perf_mode=DRM,
                            )

                    if pi < 2:
                        # head of the slot: both hi-term chains first (their
                        # weight tiles arrive before x-lo does on slot 0)
                        chain(ps1, 0, 1, 0)
                        chain(ps3, 2, 3, 0)
                        chain(ps1, 0, 1, 1)
                        chain(ps3, 2, 3, 1)
                    else:
                        chain(ps1, 0, 1, 2)
                        chain(ps3, 2, 3, 2)
                    sil = tp.tile([P, TOK], f32, tag="sil", name=f"sil_{s}_{hg}")
                    nc.scalar.activation(sil[:], ps1[:], Act.Silu, scale=1.0 / SW1)
                    h16 = tp.tile([P, TOK], f32, tag="h16", name=f"h16_{s}_{hg}")
                    nc.vector.tensor_tensor(out=h16[:], in0=sil[:], in1=ps3[:],
                                            op=AluOpType.mult)
                    nc.gpsimd.tensor_copy(out=h[:, hg, :], in_=h16[:])
                    if hg == HC - 1:  # duplicate hh10 for the leftover pair
                        nc.gpsimd.tensor_copy(out=h[:, HC, :], in_=h16[:])
                    else:  # hl10 is unused (its correction pair is dropped)
                        nc.vector.tensor_tensor(out=h[:, HC + 1 + hg, :],
                                                in0=h16[:], in1=h[:, hg, :],
                                                op=AluOpType.subtract)

                # ---- phase 2: out = (h_hi+h_lo) @ (w2_hi+w2_lo), 3 terms
                # covered by 17 DoubleRow pairs (see module docstring) ----
                HL = HC + 1  # h-lo chunk base (12)
                WL = HC + 1  # w2-lo chunk base (12); wl10 sits at 11
                # pair order: the 14 pairs whose h chunks are written by
                # mid-phase-1 come first; the 3 pairs touching the last
                # processed hid-chunk's writes (hh8/hh9 cast, hl8/hl9 sub)
                # come last, so phase 2 can start before phase 1's tail
                # elementwise ops land
                # the (hl10,Z) pair is dropped: hl10's correction is worth
                # ~4e-3 of error budget and a full DR per chain
                p2_pairs = (
                    [(2 * c, 2 * c) for c in range(4)]            # main c0-3
                    + [(2 * c, WL + 2 * c) for c in range(4)]     # w2-lo c0-3
                    + [(HL + 2 * c, 2 * c) for c in range(4)]     # h-lo c0-3
                    + [(HC - 1, HC - 1)]                          # hh10 x (wh10,wl10)
                    + [(8, 8), (8, WL + 8)]                       # main/w2-lo c4 (hh9)
                    + [(HL + 8, 8)]                               # h-lo c4 (hl9)
                )
                NEARLY = 13  # pairs with no dependency on the last hg's writes

                def p2_chain(ci, lo, hi, pso):
                    g, tk = ci // TK, ci % TK
                    for n in range(lo, hi):
                        hc, wc = p2_pairs[n]
                        nc.tensor.matmul(
                            out=pso[:],
                            lhsT=h[:, hc:hc + 2, tk * P:(tk + 1) * P],
                            rhs=w2t[g][:, wc:wc + 2, :],
                            start=(n == 0),
                            stop=(n == len(p2_pairs) - 1),
                            perf_mode=DRM,
                        )

                def p2_finish(ci, pso, o_sb):
                    g, tk = ci // TK, ci % TK
                    last = (ci == NGRP * TK - 1)
                    if g % 2 == 0:
                        o_sb[tk] = op.tile([P, 2 * NG], bf16, tag=f"o_{tk}",
                                           bufs=2, name=f"o_{s}_{g // 2}_{tk}")
                    o = o_sb[tk]
                    half = o[:, (g % 2) * NG:(g % 2 + 1) * NG]
                    rows = slice(s * TOK + tk * P, s * TOK + (tk + 1) * P)
                    if last:
                        # final chain of the slot: split the copy across two
                        # engines and store its half separately so the slot
                        # tail is as short as possible
                        nc.vector.tensor_scalar_mul(out=o[:, NG:NG + NG // 2],
                                                    in0=pso[:, 0:NG // 2],
                                                    scalar1=OSC)
                        nc.scalar.activation(o[:, NG + NG // 2:2 * NG],
                                             pso[:, NG // 2:NG], Act.Copy,
                                             scale=OSC)
                        nc.gpsimd.dma_start(
                            out=out[rows, (2 * (g // 2) + 1) * NG:
                                    (2 * (g // 2) + 2) * NG],
                            in_=o[:, NG:2 * NG],
                        )
                        return
                    if ci % 2 == 0:
                        nc.vector.tensor_scalar_mul(out=half, in0=pso[:],
                                                    scalar1=OSC)
                    else:
                        nc.scalar.activation(half, pso[:], Act.Copy, scale=OSC)
                    if g % 2 == 1:
                        nc.gpsimd.dma_start(
                            out=out[rows,
                                    (g // 2) * 2 * NG:(g // 2 + 1) * 2 * NG],
                            in_=o[:],
                        )
                    elif g == NGRP - 2 and tk == TK - 1:
                        # the last chain (g3,tk3) stores its own half; ship
                        # this g2 half now instead of pairing with it
                        nc.gpsimd.dma_start(
                            out=out[rows, (g // 2) * 2 * NG:
                                    (g // 2) * 2 * NG + NG],
                            in_=o[:, 0:NG],
                        )

                o_sb = {}
                NP2 = len(p2_pairs)
                # first three chains interleaved: their early pairs run while
                # the last hg's h writes land, then their tails complete
                first = [psb.tile([P, NG], f32, tag="ps", name=f"pso_{s}_{ci}")
                         for ci in range(3)]
                for ci in range(3):
                    p2_chain(ci, 0, NEARLY, first[ci])
                for ci in range(3):
                    p2_chain(ci, NEARLY, NP2, first[ci])
                    p2_finish(ci, first[ci], o_sb)
                for ci in range(3, NGRP * TK):
                    pso = psb.tile([P, NG], f32, tag="ps", name=f"pso_{s}_{ci}")
                    p2_chain(ci, 0, NP2, pso)
                    p2_finish(ci, pso, o_sb)
    nc.compile()
    return nc


def _plan(m_sizes, T):
    """Mirror the reference routing: contiguous segments by expert, then chop
    into TOK-sized chunks and deal them contiguously across cores."""
    bounds = np.cumsum(np.asarray(m_sizes, dtype=np.int64))
    E = len(bounds)
    chunks = []  # (expert, row_start, nrows)
    prev = 0
    for e in range(E):
        lo, hi = prev, min(int(bounds[e]), T)
        prev = max(lo, hi)
        seg = hi - lo
        off = lo
        while seg > 0:
            take = min(TOK, seg)
            chunks.append((e, off, take))
            off += take
            seg -= take
    S = max(1, math.ceil(len(chunks) / N_CORES))
    while len(chunks) < N_CORES * S:
        chunks.append((0, 0, 0))  # dummy slot
    per_core = [chunks[c * S:(c + 1) * S] for c in range(N_CORES)]
    return per_core, S


def _hilo(a):
    hi = a.astype(F8)
    lo = (a - hi.astype(np.float32)).astype(F8)
    return hi, lo


def _quant_weights(w1, w2, w3):
    """Per-expert hi/lo fp8 weights in the on-device layouts."""
    E = w1.shape[0]
    w13_e = np.empty((E, HC, P, 4, KC, P), dtype=F8)
    w2_e = np.empty((E, NGRP, P, NCW, NG), dtype=F8)

    def t13(a):  # [D, H] -> [HC, P(k), KC, P(h)]
        return a.reshape(KC, P, HC, P).transpose(2, 1, 0, 3)

    def t2(a):  # [H, D] -> [NGRP, P(h), HC, NG]
        return a.reshape(HC, P, NGRP, NG).transpose(2, 1, 0, 3)

    for e in range(E):
        h1, l1 = _hilo(w1[e] * SW1)
        h3, l3 = _hilo(w3[e] * SW3)
        w13_e[e, :, :, 0] = t13(h1)
        w13_e[e, :, :, 1] = t13(l1)
        w13_e[e, :, :, 2] = t13(h3)
        w13_e[e, :, :, 3] = t13(l3)
        h2, l2 = _hilo(w2[e] * SW2)
        th, tl = t2(h2), t2(l2)
        w2_e[e, :, :, 0:HC] = th          # wh0..wh10
        w2_e[e, :, :, HC] = tl[:, :, HC - 1]   # wl10
        w2_e[e, :, :, HC + 1:NCW] = tl[:, :, 0:HC - 1]  # wl0..wl9
    return w13_e, w2_e


def kernel(x, w1, w2, w3, m_sizes, _trace=False):
    x = np.asarray(x, dtype=np.float32)
    w1 = np.asarray(w1, dtype=np.float32)
    w2 = np.asarray(w2, dtype=np.float32)
    w3 = np.asarray(w3, dtype=np.float32)
    T = x.shape[0]
    assert x.shape[1] == D and w1.shape[1:] == (D, H), (x.shape, w1.shape)
    assert w2.shape[1:] == (H, D) and w3.shape[1:] == (D, H), (w2.shape, w3.shape)

    per_core, S = _plan(m_sizes, T)

    if S not in _compiled_cache:
        _compiled_cache[S] = _build_program(S)
    nc = _compiled_cache[S]

    wkey = (id(w1), id(w2), id(w3))
    if wkey not in _wq_cache:
        _wq_cache.clear()
        _wq_cache[wkey] = _quant_weights(w1, w2, w3)
    w13_e, w2_e = _wq_cache[wkey]

    in_maps = []
    for c in range(N_CORES):
        slots = per_core[c]
        xt_c = np.zeros((S, 2, P, KC, TOK), dtype=F8)
        for s, (e, off, ln) in enumerate(slots):
            if ln:
                seg = np.zeros((TOK, D), dtype=np.float32)
                seg[:ln] = x[off:off + ln]
                sh, sl = _hilo(seg)
                # [TOK, D] -> [P(k), KC, TOK]
                xt_c[s, 0] = sh.reshape(TOK, KC, P).transpose(2, 1, 0)
                xt_c[s, 1] = sl.reshape(TOK, KC, P).transpose(2, 1, 0)
        eids = [e for (e, _, _) in slots]
        in_maps.append({
            "xt": xt_c,
            "w13": np.ascontiguousarray(w13_e[eids]),
            "w2c": np.ascontiguousarray(w2_e[eids]),
        })

    try:
        res = run_bass_kernel_spmd(
            nc, in_maps, list(range(N_CORES)), trace=_trace,
        )
    except Exception:
        # transient NRT device errors have been observed once after a fresh
        # compile; a single retry is free if the device truly died
        res = run_bass_kernel_spmd(
            nc, in_maps, list(range(N_CORES)), trace=_trace,
        )

    full = np.zeros((T, D), dtype=np.float32)
    for c in range(N_CORES):
        oc = res.results[c]["out"].astype(np.float32)
        for s, (e, off, ln) in enumerate(per_core[c]):
            if ln:
                full[off:off + ln] = oc[s * TOK:s * TOK + ln]

    last_run_info.clear()
    last_run_info.update({
        "exec_time_ns": res.exec_time_ns,
        "profile_json": getattr(res, "profile_json", None),
        "S": S,
    })
    return full


# revision 40
# speedup vs baseline: 1.3948x; 1.0088x over previous
"""Grouped SwiGLU experts (MoE, contiguous per-expert token segments) on 8 trn2 cores.

Strategy: expert-parallel over 512-token slots (as the bf16 baseline), but all
matmuls run in fp8-e4m3 with MatmulPerfMode.DoubleRow (K=256 per instruction,
0.5 cycles/row -> 4x bf16 FLOP rate).  Plain fp8 is ~6.5% rel err, far over
the 2e-2 budget, so every tensor is carried as an exact-ish hi+lo e4m3 pair
and each GEMM computes the three significant cross terms
    a@b ~= a_hi@b_hi + a_hi@b_lo + a_lo@b_hi        (lo@lo ~ 1e-3 rel, dropped)
The lo parts keep the SAME scale as their hi parts, so all three terms
accumulate into a single PSUM bank with no fixup arithmetic (end-to-end rel
err ~3e-3 measured in numpy).  Net tensor-engine time is 0.75x rows vs 1.0x
for bf16 -> ~1.3x speedup.

Scales keep everything in e4m3's happy range: x at 1, w1 at 64, w3 at 16,
w2 at 64; h is produced at scale 16 (= silu(ps1/64) * ps3), output PSUM is
scale 1024 and is written back as bf16 after a 1/1024 scale.

H=1408 is 11 128-chunks (odd = 33 product chunks over the 3 terms), so the
phase-2 chunk layouts are arranged to cover all 33 in exactly 17 DoubleRow
pairs: h is [hh0..hh10, hh10(dup), hl0..hl10, Z] and w2 is
[wh0..wh10, wl10, wl0..wl9].  Pairs:
  5x (hh2c,hh2c+1)x(wh2c,wh2c+1)   main
  1x (hh10,hh10')x(wh10,wl10)      both hh10 leftovers in one pair
  5x (hh2c,hh2c+1)x(wl2c,wl2c+1)   w2-lo correction
  5x (hl2c,hl2c+1)x(wh2c,wh2c+1)   h-lo correction
  1x (hl10,Z)x(wh10,wl10)          last h-lo leftover

kernel(**inputs) -> full [16384, 2048] fp32 output.  Self-contained.
"""

import math

import numpy as np
import ml_dtypes

import concourse.bass as bass
import concourse.tile as tile
from concourse import bacc
from concourse import mybir
from concourse.alu_op_type import AluOpType
from concourse.bass_utils import run_bass_kernel_spmd

N_CORES = 8
D = 2048          # dim_in
H = 1408          # dim_hidden
TOK = 512         # tokens per slot
P = 128           # partitions
KC = D // P       # 16 k-chunks over dim_in
HC = H // P       # 11 hid-chunks
TK = TOK // P     # 4 token tiles per slot
NG = 512          # phase-2 out-column group width
NGRP = D // NG    # 4 column groups
NCH = 24          # h chunk axis: [hh*11, hh10dup, hl*11, Z]
NCW = 22          # w2 chunk axis: [wh*11, wl10, wl0..wl9]
WR = 5            # w13 tag rotation depth (SBUF vs prefetch tradeoff)

SW1 = 64.0        # w1 quant scale
SW3 = 16.0        # w3 quant scale -> h comes out at scale 16
SW2 = 64.0        # w2 quant scale
OSC = 1.0 / (16.0 * 64.0)  # final psum -> out scale

F8 = ml_dtypes.float8_e4m3

_compiled_cache = {}
_wq_cache = {}
last_run_info = {}


def _build_program(S):
    """Per-core SPMD program: S slots, each 512 tokens of one expert."""
    nc = bacc.Bacc()
    f8 = mybir.dt.float8e4
    f32 = mybir.dt.float32
    bf16 = mybir.dt.bfloat16
    DRM = mybir.MatmulPerfMode.DoubleRow
    Act = mybir.ActivationFunctionType

    xt = nc.declare_dram_parameter("xt", [S, 2, P, KC, TOK], f8, isOutput=False)
    w13 = nc.declare_dram_parameter("w13", [S, HC, P, 4, KC, P], f8, isOutput=False)
    w2c = nc.declare_dram_parameter("w2c", [S, NGRP, P, NCW, NG], f8, isOutput=False)
    out = nc.declare_dram_parameter("out", [S * TOK, D], bf16, isOutput=True)

    with tile.TileContext(nc) as tc:
        with (
            tc.tile_pool(name="xp", bufs=2) as xp,
            tc.tile_pool(name="wp", bufs=1) as wp,
            tc.tile_pool(name="w2p", bufs=1) as w2p,
            tc.tile_pool(name="hp", bufs=2) as hp,
            tc.tile_pool(name="tp", bufs=2) as tp,
            tc.tile_pool(name="op", bufs=4) as op,
            tc.tile_pool(name="psA", bufs=5, space="PSUM") as psa,
            tc.tile_pool(name="psB", bufs=3, space="PSUM") as psb,
        ):
            # phase-1 processing order: hg10 runs mid-phase so its h chunks
            # (hh10, the dup, hl10) are long done before phase 2 reads them
            # in its final DoubleRow pairs
            PROC = [0, 1, 2, 3, 4, 10, 5, 6, 7, 8, 9]

            # PE pstate warmup: ~150 tiny self-contained matmuls on scratch
            # data keep the PE busy through the initial DMA wait so the
            # first real chains run at full clock
            scr_w = tp.tile([P, 2, P], f8, tag="scrw", bufs=1, name="scr_w")
            scr_x = tp.tile([P, 2, 64], f8, tag="scrx", bufs=1, name="scr_x")
            nc.gpsimd.memset(scr_w[:], 0)
            nc.gpsimd.memset(scr_x[:], 0)
            scr_ps = psa.tile([P, TOK], f32, tag="ps", name="scr_ps")
            for i in range(150):
                nc.tensor.matmul(out=scr_ps[:, 0:64], lhsT=scr_w[:], rhs=scr_x[:],
                                 start=True, stop=True, perf_mode=DRM)

            for s in range(S):
                # ---- loads.  All on the SP queue, which is a strict
                # blocking FIFO (a DMA holds the SEQ during its semaphore
                # waits), so issue order IS priority order.  w13 issues are
                # paced by their tag-rotation frees; by the time the FIFO
                # reaches this slot's w2 issues, the previous slot's phase 2
                # is done, so they never block later loads. ----
                xh = xp.tile([P, KC, TOK], f8, tag="xh", name=f"xh_{s}")
                xl = xp.tile([P, KC, TOK], f8, tag="xl", name=f"xl_{s}")
                wt = {}

                def load_w13(pos, s=s, wt=wt):
                    hg = PROC[pos]
                    t = wp.tile([P, 4, KC, P], f8, tag=f"w_{pos % WR}",
                                name=f"w13_{s}_{hg}")
                    nc.sync.dma_start(out=t[:], in_=w13[s, hg])
                    wt[hg] = t

                KH = KC // 2
                if s == 0:
                    # fine-grained first loads: the first chain can start
                    # after one x quarter + half a w13 kind instead of 2MB
                    t = wp.tile([P, 4, KC, P], f8, tag="w_0", name="w13_0_0")
                    wt[0] = t
                    nc.sync.dma_start(out=t[:, 0, 0:KH, :], in_=w13[0, 0, :, 0, 0:KH])
                    nc.sync.dma_start(out=xh[:, 0:4, :], in_=xt[0, 0, :, 0:4])
                    nc.sync.dma_start(out=t[:, 0, KH:KC, :], in_=w13[0, 0, :, 0, KH:KC])
                    nc.sync.dma_start(out=xh[:, 4:8, :], in_=xt[0, 0, :, 4:8])
                    nc.sync.dma_start(out=t[:, 1], in_=w13[0, 0, :, 1])
                    nc.sync.dma_start(out=xh[:, 8:12, :], in_=xt[0, 0, :, 8:12])
                    nc.sync.dma_start(out=xh[:, 12:16, :], in_=xt[0, 0, :, 12:16])
                    nc.sync.dma_start(out=t[:, 2], in_=w13[0, 0, :, 2])
                    nc.sync.dma_start(out=t[:, 3], in_=w13[0, 0, :, 3])
                    # second tile's w1 kinds land before x-lo so hg1 can
                    # start the moment hg0's x-lo terms finish
                    t1 = wp.tile([P, 4, KC, P], f8, tag="w_1", name="w13_0_1")
                    wt[1] = t1
                    nc.sync.dma_start(out=t1[:, 0:2], in_=w13[0, 1, :, 0:2])
                    # x-lo chunks 14,15 are never used (skipped correction)
                    nc.sync.dma_start(out=xl[:, 0:KH, :], in_=xt[0, 1, :, 0:KH])
                    nc.sync.dma_start(out=xl[:, KH:KC - 2, :],
                                      in_=xt[0, 1, :, KH:KC - 2])
                    nc.sync.dma_start(out=t1[:, 2:4], in_=w13[0, 1, :, 2:4])
                else:
                    nc.sync.dma_start(out=xh[:, 0:KH, :], in_=xt[s, 0, :, 0:KH])
                    load_w13(0)
                    nc.sync.dma_start(out=xh[:, KH:KC, :], in_=xt[s, 0, :, KH:KC])
                    nc.sync.dma_start(out=xl[:, 0:KH, :], in_=xt[s, 1, :, 0:KH])
                    nc.sync.dma_start(out=xl[:, KH:KC - 2, :],
                                      in_=xt[s, 1, :, KH:KC - 2])
                for pos in range(2 if s == 0 else 1, HC):
                    load_w13(pos)
                w2t = []
                for g in range(NGRP):
                    t = w2p.tile([P, NCW, NG], f8, tag=f"w2_{g}",
                                 name=f"w2_{s}_{g}")
                    nc.sync.dma_start(out=t[:], in_=w2c[s, g])
                    w2t.append(t)

                h = hp.tile([P, NCH, TOK], f8, tag="h", name=f"h_{s}")

                # ---- phase 1: h = silu(x@w1) * (x@w3), hi/lo split on chip ----
                for pi, hg in enumerate(PROC):
                    ps1 = psa.tile([P, TOK], f32, tag="ps", name=f"ps1_{s}_{hg}")
                    ps3 = psa.tile([P, TOK], f32, tag="ps", name=f"ps3_{s}_{hg}")
                    w = wt[hg]

                    # x-lo correction covers K chunks 0..13 only (the last
                    # DoubleRow pair is skipped: spends ~1e-2 of the 2e-2
                    # error budget for ~9us)
                    def chain(psx, khi, klo, part):
                        seq = ([(khi, xh, d, n == 0, False)
                                for n, d in enumerate(range(KC // 2))]
                               + [(klo, xh, d, False, False)
                                  for d in range(KC // 2)]
                               + [(khi, xl, d, False, d == KC // 2 - 2)
                                  for d in range(KC // 2 - 1)])
                        lo, hi = (0, 2 * (KC // 2)) if part == 0 else \
                                 (2 * (KC // 2), len(seq)) if part == 1 else \
                                 (0, len(seq))
                        for wk, xx, d, st, sp in seq[lo:hi]:
                            nc.tensor.matmul(
                                out=psx[:], lhsT=w[:, wk, 2 * d:2 * d + 2, :],
                                rhs=xx[:, 2 * d:2 * d + 2, :],
                                start=st, stop=sp, perf_mode=DRM,
                            )

                    if pi < 2:
                        # head of the slot: both hi-term chains first (their
                        # weight tiles arrive before x-lo does on slot 0)
                        chain(ps1, 0, 1, 0)
                        chain(ps3, 2, 3, 0)
                        chain(ps1, 0, 1, 1)
                        chain(ps3, 2, 3, 1)
                    else:
                        chain(ps1, 0, 1, 2)
                        chain(ps3, 2, 3, 2)
                    sil = tp.tile([P, TOK], f32, tag="sil", name=f"sil_{s}_{hg}")
                    nc.scalar.activation(sil[:], ps1[:], Act.Silu, scale=1.0 / SW1)
                    h16 = tp.tile([P, TOK], f32, tag="h16", name=f"h16_{s}_{hg}")
                    nc.vector.tensor_tensor(out=h16[:], in0=sil[:], in1=ps3[:],
                                            op=AluOpType.mult)
                    nc.gpsimd.tensor_copy(out=h[:, hg, :], in_=h16[:])
                    if hg == HC - 1:  # duplicate hh10 for the leftover pair
                        nc.gpsimd.tensor_copy(out=h[:, HC, :], in_=h16[:])
                    else:  # hl10 is unused (its correction pair is dropped)
                        nc.vector.tensor_tensor(out=h[:, HC + 1 + hg, :],
                                                in0=h16[:], in1=h[:, hg, :],
                                                op=AluOpType.subtract)

                # ---- phase 2: out = (h_hi+h_lo) @ (w2_hi+w2_lo), 3 terms
                # covered by 17 DoubleRow pairs (see module docstring) ----
                HL = HC + 1  # h-lo chunk base (12)
                WL = HC + 1  # w2-lo chunk base (12); wl10 sits at 11
                # pair order: the 14 pairs whose h chunks are written by
                # mid-phase-1 come first; the 3 pairs touching the last
                # processed hid-chunk's writes (hh8/hh9 cast, hl8/hl9 sub)
                # come last, so phase 2 can start before phase 1's tail
                # elementwise ops land
                # the (hl10,Z) pair is dropped: hl10's correction is worth
                # ~4e-3 of error budget and a full DR per chain
                p2_pairs = (
                    [(2 * c, 2 * c) for c in range(4)]            # main c0-3
                    + [(2 * c, WL + 2 * c) for c in range(4)]     # w2-lo c0-3
                    + [(HL + 2 * c, 2 * c) for c in range(4)]     # h-lo c0-3
                    + [(HC - 1, HC - 1)]                          # hh10 x (wh10,wl10)
                    + [(8, 8), (8, WL + 8)]                       # main/w2-lo c4 (hh9)
                    + [(HL + 8, 8)]                               # h-lo c4 (hl9)
                )
                NEARLY = 13  # pairs with no dependency on the last hg's writes

                def p2_chain(ci, lo, hi, pso):
                    g, tk = ci // TK, ci % TK
                    for n in range(lo, hi):
                        hc, wc = p2_pairs[n]
                        nc.tensor.matmul(
                            out=pso[:],
                            lhsT=h[:, hc:hc + 2, tk * P:(tk + 1) * P],
                            rhs=w2t[g][:, wc:wc + 2, :],
                            start=(n == 0),
                            stop=(n == len(p2_pairs) - 1),
                            perf_mode=DRM,
                        )

                def p2_finish(ci, pso, o_sb):
                    g, tk = ci // TK, ci % TK
                    last = (ci == NGRP * TK - 1)
                    if g % 2 == 0:
                        o_sb[tk] = op.tile([P, 2 * NG], bf16, tag=f"o_{tk}",
                                           bufs=2, name=f"o_{s}_{g // 2}_{tk}")
                    o = o_sb[tk]
                    half = o[:, (g % 2) * NG:(g % 2 + 1) * NG]
                    rows = slice(s * TOK + tk * P, s * TOK + (tk + 1) * P)
                    if last:
                        # final chain of the slot: copy AND store both via
                        # the Act queue — same-engine ordering needs no
                        # cross-engine semaphore hops, shortening the tail
                        nc.scalar.activation(o[:, NG:2 * NG], pso[:],
                                             Act.Copy, scale=OSC)
                        nc.scalar.dma_start(
                            out=out[rows, (2 * (g // 2) + 1) * NG:
                                    (2 * (g // 2) + 2) * NG],
                            in_=o[:, NG:2 * NG],
                        )
                        return
                    if ci % 2 == 0:
                        nc.vector.tensor_scalar_mul(out=half, in0=pso[:],
                                                    scalar1=OSC)
                    else:
                        nc.scalar.activation(half, pso[:], Act.Copy, scale=OSC)
                    if g % 2 == 1:
                        nc.gpsimd.dma_start(
                            out=out[rows,
                                    (g // 2) * 2 * NG:(g // 2 + 1) * 2 * NG],
                            in_=o[:],
                        )
                    elif g == NGRP - 2 and tk == TK - 1:
                        # the last chain (g3,tk3) stores its own half; ship
                        # this g2 half now instead of pairing with it
                        nc.gpsimd.dma_start(
                            out=out[rows, (g // 2) * 2 * NG:
                                    (g // 2) * 2 * NG + NG],
                            in_=o[:, 0:NG],
                        )

                o_sb = {}
                NP2 = len(p2_pairs)
                # first three chains interleaved: their early pairs run while
                # the last hg's h writes land, then their tails complete
                first = [psb.tile([P, NG], f32, tag="ps", name=f"pso_{s}_{ci}")
                         for ci in range(3)]
                for ci in range(3):
                    p2_chain(ci, 0, NEARLY, first[ci])
                for ci in range(3):
                    p2_chain(ci, NEARLY, NP2, first[ci])
                    p2_finish(ci, first[ci], o_sb)
                for ci in range(3, NGRP * TK):
                    pso = psb.tile([P, NG], f32, tag="ps", name=f"pso_{s}_{ci}")
                    p2_chain(ci, 0, NP2, pso)
                    p2_finish(ci, pso, o_sb)
    nc.compile()
    return nc


def _plan(m_sizes, T):
    """Mirror the reference routing: contiguous segments by expert, then chop
    into TOK-sized chunks and deal them contiguously across cores."""
    bounds = np.cumsum(np.asarray(m_sizes, dtype=np.int64))
    E = len(bounds)
    chunks = []  # (expert, row_start, nrows)
    prev = 0
    for e in range(E):
        lo, hi = prev, min(int(bounds[e]), T)
        prev = max(lo, hi)
        seg = hi - lo
        off = lo
        while seg > 0:
            take = min(TOK, seg)
            chunks.append((e, off, take))
            off += take
            seg -= take
    S = max(1, math.ceil(len(chunks) / N_CORES))
    while len(chunks) < N_CORES * S:
        chunks.append((0, 0, 0))  # dummy slot
    per_core = [chunks[c * S:(c + 1) * S] for c in range(N_CORES)]
    return per_core, S


def _hilo(a):
    hi = a.astype(F8)
    lo = (a - hi.astype(np.float32)).astype(F8)
    return hi, lo


def _quant_weights(w1, w2, w3):
    """Per-expert hi/lo fp8 weights in the on-device layouts."""
    E = w1.shape[0]
    w13_e = np.empty((E, HC, P, 4, KC, P), dtype=F8)
    w2_e = np.empty((E, NGRP, P, NCW, NG), dtype=F8)

    def t13(a):  # [D, H] -> [HC, P(k), KC, P(h)]
        return a.reshape(KC, P, HC, P).transpose(2, 1, 0, 3)

    def t2(a):  # [H, D] -> [NGRP, P(h), HC, NG]
        return a.reshape(HC, P, NGRP, NG).transpose(2, 1, 0, 3)

    for e in range(E):
        h1, l1 = _hilo(w1[e] * SW1)
        h3, l3 = _hilo(w3[e] * SW3)
        w13_e[e, :, :, 0] = t13(h1)
        w13_e[e, :, :, 1] = t13(l1)
        w13_e[e, :, :, 2] = t13(h3)
        w13_e[e, :, :, 3] = t13(l3)
        h2, l2 = _hilo(w2[e] * SW2)
        th, tl = t2(h2), t2(l2)
        w2_e[e, :, :, 0:HC] = th          # wh0..wh10
        w2_e[e, :, :, HC] = tl[:, :, HC - 1]   # wl10
        w2_e[e, :, :, HC + 1:NCW] = tl[:, :, 0:HC - 1]  # wl0..wl9
    return w13_e, w2_e


def kernel(x, w1, w2, w3, m_sizes, _trace=False):
    x = np.asarray(x, dtype=np.float32)
    w1 = np.asarray(w1, dtype=np.float32)
    w2 = np.asarray(w2, dtype=np.float32)
    w3 = np.asarray(w3, dtype=np.float32)
    T = x.shape[0]
    assert x.shape[1] == D and w1.shape[1:] == (D, H), (x.shape, w1.shape)
    assert w2.shape[1:] == (H, D) and w3.shape[1:] == (D, H), (w2.shape, w3.shape)

    per_core, S = _plan(m_sizes, T)

    if S not in _compiled_cache:
        _compiled_cache[S] = _build_program(S)
    nc = _compiled_cache[S]

    wkey = (id(w1), id(w2), id(w3))
    if wkey not in _wq_cache:
        _wq_cache.clear()
        _wq_cache[wkey] = _quant_weights(w1, w2, w3)
    w13_e, w2_e = _wq_cache[wkey]

    in_maps = []
    for c in range(N_CORES):
        slots = per_core[c]
        xt_c = np.zeros((S, 2, P, KC, TOK), dtype=F8)
        for s, (e, off, ln) in enumerate(slots):
            if ln:
                seg = np.zeros((TOK, D), dtype=np.float32)
                seg[:ln] = x[off:off + ln]
                sh, sl = _hilo(seg)
                # [TOK, D] -> [P(k), KC, TOK]
                xt_c[s, 0] = sh.reshape(TOK, KC, P).transpose(2, 1, 0)
                xt_c[s, 1] = sl.reshape(TOK, KC, P).transpose(2, 1, 0)
        eids = [e for (e, _, _) in slots]
        in_maps.append({
            "xt": xt_c,
            "w13": np.ascontiguousarray(w13_e[eids]),
            "w2c": np.ascontiguousarray(w2_e[eids]),
        })

    try:
        res = run_bass_kernel_spmd(
            nc, in_maps, list(range(N_CORES)), trace=_trace,
        )
    except Exception:
        # transient NRT device errors have been observed once after a fresh
        # compile; a single retry is free if the device truly died
        res = run_bass_kernel_spmd(
            nc, in_maps, list(range(N_CORES)), trace=_trace,
        )

    full = np.zeros((T, D), dtype=np.float32)
    for c in range(N_CORES):
        oc = res.results[c]["out"].astype(np.float32)
        for s, (e, off, ln) in enumerate(per_core[c]):
            if ln:
                full[off:off + ln] = oc[s * TOK:s * TOK + ln]

    last_run_info.clear()
    last_run_info.update({
        "exec_time_ns": res.exec_time_ns,
        "profile_json": getattr(res, "profile_json", None),
        "S": S,
    })
    return full


# revision 41
# speedup vs baseline: 1.3989x; 1.0029x over previous
"""Grouped SwiGLU experts (MoE, contiguous per-expert token segments) on 8 trn2 cores.

Strategy: expert-parallel over 512-token slots (as the bf16 baseline), but all
matmuls run in fp8-e4m3 with MatmulPerfMode.DoubleRow (K=256 per instruction,
0.5 cycles/row -> 4x bf16 FLOP rate).  Plain fp8 is ~6.5% rel err, far over
the 2e-2 budget, so every tensor is carried as an exact-ish hi+lo e4m3 pair
and each GEMM computes the three significant cross terms
    a@b ~= a_hi@b_hi + a_hi@b_lo + a_lo@b_hi        (lo@lo ~ 1e-3 rel, dropped)
The lo parts keep the SAME scale as their hi parts, so all three terms
accumulate into a single PSUM bank with no fixup arithmetic (end-to-end rel
err ~3e-3 measured in numpy).  Net tensor-engine time is 0.75x rows vs 1.0x
for bf16 -> ~1.3x speedup.

Scales keep everything in e4m3's happy range: x at 1, w1 at 64, w3 at 16,
w2 at 64; h is produced at scale 16 (= silu(ps1/64) * ps3), output PSUM is
scale 1024 and is written back as bf16 after a 1/1024 scale.

H=1408 is 11 128-chunks (odd = 33 product chunks over the 3 terms), so the
phase-2 chunk layouts are arranged to cover all 33 in exactly 17 DoubleRow
pairs: h is [hh0..hh10, hh10(dup), hl0..hl10, Z] and w2 is
[wh0..wh10, wl10, wl0..wl9].  Pairs:
  5x (hh2c,hh2c+1)x(wh2c,wh2c+1)   main
  1x (hh10,hh10')x(wh10,wl10)      both hh10 leftovers in one pair
  5x (hh2c,hh2c+1)x(wl2c,wl2c+1)   w2-lo correction
  5x (hl2c,hl2c+1)x(wh2c,wh2c+1)   h-lo correction
  1x (hl10,Z)x(wh10,wl10)          last h-lo leftover

kernel(**inputs) -> full [16384, 2048] fp32 output.  Self-contained.
"""

import math

import numpy as np
import ml_dtypes

import concourse.bass as bass
import concourse.tile as tile
from concourse import bacc
from concourse import mybir
from concourse.alu_op_type import AluOpType
from concourse.bass_utils import run_bass_kernel_spmd

N_CORES = 8
D = 2048          # dim_in
H = 1408          # dim_hidden
TOK = 512         # tokens per slot
P = 128           # partitions
KC = D // P       # 16 k-chunks over dim_in
HC = H // P       # 11 hid-chunks
TK = TOK // P     # 4 token tiles per slot
NG = 512          # phase-2 out-column group width
NGRP = D // NG    # 4 column groups
NCH = 24          # h chunk axis: [hh*11, hh10dup, hl*11, Z]
NCW = 22          # w2 chunk axis: [wh*11, wl10, wl0..wl9]
WR = 5            # w13 tag rotation depth (SBUF vs prefetch tradeoff)

SW1 = 64.0        # w1 quant scale
SW3 = 16.0        # w3 quant scale -> h comes out at scale 16
SW2 = 64.0        # w2 quant scale
OSC = 1.0 / (16.0 * 64.0)  # final psum -> out scale

F8 = ml_dtypes.float8_e4m3

_compiled_cache = {}
_wq_cache = {}
last_run_info = {}


def _build_program(S):
    """Per-core SPMD program: S slots, each 512 tokens of one expert."""
    nc = bacc.Bacc()
    f8 = mybir.dt.float8e4
    f32 = mybir.dt.float32
    bf16 = mybir.dt.bfloat16
    DRM = mybir.MatmulPerfMode.DoubleRow
    Act = mybir.ActivationFunctionType

    xt = nc.declare_dram_parameter("xt", [S, 2, P, KC, TOK], f8, isOutput=False)
    w13 = nc.declare_dram_parameter("w13", [S, HC, P, 4, KC, P], f8, isOutput=False)
    w2c = nc.declare_dram_parameter("w2c", [S, NGRP, P, NCW, NG], f8, isOutput=False)
    out = nc.declare_dram_parameter("out", [S * TOK, D], bf16, isOutput=True)

    with tile.TileContext(nc) as tc:
        with (
            tc.tile_pool(name="xp", bufs=2) as xp,
            tc.tile_pool(name="wp", bufs=1) as wp,
            tc.tile_pool(name="w2p", bufs=1) as w2p,
            tc.tile_pool(name="hp", bufs=2) as hp,
            tc.tile_pool(name="tp", bufs=2) as tp,
            tc.tile_pool(name="op", bufs=4) as op,
            tc.tile_pool(name="psA", bufs=4, space="PSUM") as psa,
            tc.tile_pool(name="psB", bufs=4, space="PSUM") as psb,
        ):
            # phase-1 processing order: hg10 runs mid-phase so its h chunks
            # (hh10, the dup, hl10) are long done before phase 2 reads them
            # in its final DoubleRow pairs
            PROC = [0, 1, 2, 3, 4, 10, 5, 6, 7, 8, 9]

            # PE pstate warmup: ~150 tiny self-contained matmuls on scratch
            # data keep the PE busy through the initial DMA wait so the
            # first real chains run at full clock
            scr_w = tp.tile([P, 2, P], f8, tag="scrw", bufs=1, name="scr_w")
            scr_x = tp.tile([P, 2, 64], f8, tag="scrx", bufs=1, name="scr_x")
            nc.gpsimd.memset(scr_w[:], 0)
            nc.gpsimd.memset(scr_x[:], 0)
            scr_ps = psa.tile([P, TOK], f32, tag="ps", name="scr_ps")
            for i in range(150):
                nc.tensor.matmul(out=scr_ps[:, 0:64], lhsT=scr_w[:], rhs=scr_x[:],
                                 start=True, stop=True, perf_mode=DRM)

            for s in range(S):
                # ---- loads.  All on the SP queue, which is a strict
                # blocking FIFO (a DMA holds the SEQ during its semaphore
                # waits), so issue order IS priority order.  w13 issues are
                # paced by their tag-rotation frees; by the time the FIFO
                # reaches this slot's w2 issues, the previous slot's phase 2
                # is done, so they never block later loads. ----
                xh = xp.tile([P, KC, TOK], f8, tag="xh", name=f"xh_{s}")
                xl = xp.tile([P, KC, TOK], f8, tag="xl", name=f"xl_{s}")
                wt = {}

                def load_w13(pos, s=s, wt=wt):
                    hg = PROC[pos]
                    t = wp.tile([P, 4, KC, P], f8, tag=f"w_{pos % WR}",
                                name=f"w13_{s}_{hg}")
                    nc.sync.dma_start(out=t[:], in_=w13[s, hg])
                    wt[hg] = t

                KH = KC // 2
                if s == 0:
                    # fine-grained first loads: the first chain can start
                    # after one x quarter + half a w13 kind instead of 2MB
                    t = wp.tile([P, 4, KC, P], f8, tag="w_0", name="w13_0_0")
                    wt[0] = t
                    nc.sync.dma_start(out=t[:, 0, 0:KH, :], in_=w13[0, 0, :, 0, 0:KH])
                    nc.sync.dma_start(out=xh[:, 0:4, :], in_=xt[0, 0, :, 0:4])
                    nc.sync.dma_start(out=t[:, 0, KH:KC, :], in_=w13[0, 0, :, 0, KH:KC])
                    nc.sync.dma_start(out=xh[:, 4:8, :], in_=xt[0, 0, :, 4:8])
                    nc.sync.dma_start(out=t[:, 1], in_=w13[0, 0, :, 1])
                    nc.sync.dma_start(out=xh[:, 8:12, :], in_=xt[0, 0, :, 8:12])
                    nc.sync.dma_start(out=xh[:, 12:16, :], in_=xt[0, 0, :, 12:16])
                    nc.sync.dma_start(out=t[:, 2], in_=w13[0, 0, :, 2])
                    nc.sync.dma_start(out=t[:, 3], in_=w13[0, 0, :, 3])
                    # second tile's w1 kinds land before x-lo so hg1 can
                    # start the moment hg0's x-lo terms finish
                    t1 = wp.tile([P, 4, KC, P], f8, tag="w_1", name="w13_0_1")
                    wt[1] = t1
                    nc.sync.dma_start(out=t1[:, 0:2], in_=w13[0, 1, :, 0:2])
                    # x-lo chunks 14,15 are never used (skipped correction)
                    nc.sync.dma_start(out=xl[:, 0:KH, :], in_=xt[0, 1, :, 0:KH])
                    nc.sync.dma_start(out=xl[:, KH:KC - 2, :],
                                      in_=xt[0, 1, :, KH:KC - 2])
                    nc.sync.dma_start(out=t1[:, 2:4], in_=w13[0, 1, :, 2:4])
                else:
                    nc.sync.dma_start(out=xh[:, 0:KH, :], in_=xt[s, 0, :, 0:KH])
                    load_w13(0)
                    nc.sync.dma_start(out=xh[:, KH:KC, :], in_=xt[s, 0, :, KH:KC])
                    nc.sync.dma_start(out=xl[:, 0:KH, :], in_=xt[s, 1, :, 0:KH])
                    nc.sync.dma_start(out=xl[:, KH:KC - 2, :],
                                      in_=xt[s, 1, :, KH:KC - 2])
                for pos in range(2 if s == 0 else 1, HC):
                    load_w13(pos)
                w2t = []
                for g in range(NGRP):
                    t = w2p.tile([P, NCW, NG], f8, tag=f"w2_{g}",
                                 name=f"w2_{s}_{g}")
                    nc.sync.dma_start(out=t[:], in_=w2c[s, g])
                    w2t.append(t)

                h = hp.tile([P, NCH, TOK], f8, tag="h", name=f"h_{s}")

                # ---- phase 1: h = silu(x@w1) * (x@w3), hi/lo split on chip ----
                for pi, hg in enumerate(PROC):
                    ps1 = psa.tile([P, TOK], f32, tag="ps", name=f"ps1_{s}_{hg}")
                    ps3 = psa.tile([P, TOK], f32, tag="ps", name=f"ps3_{s}_{hg}")
                    w = wt[hg]

                    # x-lo correction covers K chunks 0..13 only (the last
                    # DoubleRow pair is skipped: spends ~1e-2 of the 2e-2
                    # error budget for ~9us)
                    def chain(psx, khi, klo, part):
                        seq = ([(khi, xh, d, n == 0, False)
                                for n, d in enumerate(range(KC // 2))]
                               + [(klo, xh, d, False, False)
                                  for d in range(KC // 2)]
                               + [(khi, xl, d, False, d == KC // 2 - 2)
                                  for d in range(KC // 2 - 1)])
                        lo, hi = (0, 2 * (KC // 2)) if part == 0 else \
                                 (2 * (KC // 2), len(seq)) if part == 1 else \
                                 (0, len(seq))
                        for wk, xx, d, st, sp in seq[lo:hi]:
                            nc.tensor.matmul(
                                out=psx[:], lhsT=w[:, wk, 2 * d:2 * d + 2, :],
                                rhs=xx[:, 2 * d:2 * d + 2, :],
                                start=st, stop=sp, perf_mode=DRM,
                            )

                    if pi < 2:
                        # head of the slot: both hi-term chains first (their
                        # weight tiles arrive before x-lo does on slot 0)
                        chain(ps1, 0, 1, 0)
                        chain(ps3, 2, 3, 0)
                        chain(ps1, 0, 1, 1)
                        chain(ps3, 2, 3, 1)
                    else:
                        chain(ps1, 0, 1, 2)
                        chain(ps3, 2, 3, 2)
                    sil = tp.tile([P, TOK], f32, tag="sil", name=f"sil_{s}_{hg}")
                    nc.scalar.activation(sil[:], ps1[:], Act.Silu, scale=1.0 / SW1)
                    h16 = tp.tile([P, TOK], f32, tag="h16", name=f"h16_{s}_{hg}")
                    nc.vector.tensor_tensor(out=h16[:], in0=sil[:], in1=ps3[:],
                                            op=AluOpType.mult)
                    nc.gpsimd.tensor_copy(out=h[:, hg, :], in_=h16[:])
                    if hg == HC - 1:  # duplicate hh10 for the leftover pair
                        nc.gpsimd.tensor_copy(out=h[:, HC, :], in_=h16[:])
                    else:  # hl10 is unused (its correction pair is dropped)
                        nc.vector.tensor_tensor(out=h[:, HC + 1 + hg, :],
                                                in0=h16[:], in1=h[:, hg, :],
                                                op=AluOpType.subtract)

                # ---- phase 2: out = (h_hi+h_lo) @ (w2_hi+w2_lo), 3 terms
                # covered by 17 DoubleRow pairs (see module docstring) ----
                HL = HC + 1  # h-lo chunk base (12)
                WL = HC + 1  # w2-lo chunk base (12); wl10 sits at 11
                # pair order: the 14 pairs whose h chunks are written by
                # mid-phase-1 come first; the 3 pairs touching the last
                # processed hid-chunk's writes (hh8/hh9 cast, hl8/hl9 sub)
                # come last, so phase 2 can start before phase 1's tail
                # elementwise ops land
                # the (hl10,Z) pair is dropped: hl10's correction is worth
                # ~4e-3 of error budget and a full DR per chain
                p2_pairs = (
                    [(2 * c, 2 * c) for c in range(4)]            # main c0-3
                    + [(2 * c, WL + 2 * c) for c in range(4)]     # w2-lo c0-3
                    + [(HL + 2 * c, 2 * c) for c in range(4)]     # h-lo c0-3
                    + [(HC - 1, HC - 1)]                          # hh10 x (wh10,wl10)
                    + [(8, 8), (8, WL + 8)]                       # main/w2-lo c4 (hh9)
                    + [(HL + 8, 8)]                               # h-lo c4 (hl9)
                )
                NEARLY = 13  # pairs with no dependency on the last hg's writes

                def p2_chain(ci, lo, hi, pso):
                    g, tk = ci // TK, ci % TK
                    for n in range(lo, hi):
                        hc, wc = p2_pairs[n]
                        nc.tensor.matmul(
                            out=pso[:],
                            lhsT=h[:, hc:hc + 2, tk * P:(tk + 1) * P],
                            rhs=w2t[g][:, wc:wc + 2, :],
                            start=(n == 0),
                            stop=(n == len(p2_pairs) - 1),
                            perf_mode=DRM,
                        )

                def p2_finish(ci, pso, o_sb):
                    g, tk = ci // TK, ci % TK
                    last = (ci == NGRP * TK - 1)
                    if g % 2 == 0:
                        o_sb[tk] = op.tile([P, 2 * NG], bf16, tag=f"o_{tk}",
                                           bufs=2, name=f"o_{s}_{g // 2}_{tk}")
                    o = o_sb[tk]
                    half = o[:, (g % 2) * NG:(g % 2 + 1) * NG]
                    rows = slice(s * TOK + tk * P, s * TOK + (tk + 1) * P)
                    if last:
                        # final chain of the slot: copy AND store both via
                        # the Act queue — same-engine ordering needs no
                        # cross-engine semaphore hops, shortening the tail
                        nc.scalar.activation(o[:, NG:2 * NG], pso[:],
                                             Act.Copy, scale=OSC)
                        nc.scalar.dma_start(
                            out=out[rows, (2 * (g // 2) + 1) * NG:
                                    (2 * (g // 2) + 2) * NG],
                            in_=o[:, NG:2 * NG],
                        )
                        return
                    if ci % 2 == 0:
                        nc.vector.tensor_scalar_mul(out=half, in0=pso[:],
                                                    scalar1=OSC)
                    else:
                        nc.scalar.activation(half, pso[:], Act.Copy, scale=OSC)
                    if g % 2 == 1:
                        nc.gpsimd.dma_start(
                            out=out[rows,
                                    (g // 2) * 2 * NG:(g // 2 + 1) * 2 * NG],
                            in_=o[:],
                        )
                    elif g == NGRP - 2 and tk == TK - 1:
                        # the last chain (g3,tk3) stores its own half; ship
                        # this g2 half now instead of pairing with it
                        nc.gpsimd.dma_start(
                            out=out[rows, (g // 2) * 2 * NG:
                                    (g // 2) * 2 * NG + NG],
                            in_=o[:, 0:NG],
                        )

                o_sb = {}
                NP2 = len(p2_pairs)
                # first three chains interleaved: their early pairs run while
                # the last hg's h writes land, then their tails complete
                first = [psb.tile([P, NG], f32, tag="ps", name=f"pso_{s}_{ci}")
                         for ci in range(3)]
                for ci in range(3):
                    p2_chain(ci, 0, NEARLY, first[ci])
                for ci in range(3):
                    p2_chain(ci, NEARLY, NP2, first[ci])
                    p2_finish(ci, first[ci], o_sb)
                for ci in range(3, NGRP * TK):
                    pso = psb.tile([P, NG], f32, tag="ps", name=f"pso_{s}_{ci}")
                    p2_chain(ci, 0, NP2, pso)
                    p2_finish(ci, pso, o_sb)
    nc.compile()
    return nc


def _plan(m_sizes, T):
    """Mirror the reference routing: contiguous segments by expert, then chop
    into TOK-sized chunks and deal them contiguously across cores."""
    bounds = np.cumsum(np.asarray(m_sizes, dtype=np.int64))
    E = len(bounds)
    chunks = []  # (expert, row_start, nrows)
    prev = 0
    for e in range(E):
        lo, hi = prev, min(int(bounds[e]), T)
        prev = max(lo, hi)
        seg = hi - lo
        off = lo
        while seg > 0:
            take = min(TOK, seg)
            chunks.append((e, off, take))
            off += take
            seg -= take
    S = max(1, math.ceil(len(chunks) / N_CORES))
    while len(chunks) < N_CORES * S:
        chunks.append((0, 0, 0))  # dummy slot
    per_core = [chunks[c * S:(c + 1) * S] for c in range(N_CORES)]
    return per_core, S


def _hilo(a):
    hi = a.astype(F8)
    lo = (a - hi.astype(np.float32)).astype(F8)
    return hi, lo


def _quant_weights(w1, w2, w3):
    """Per-expert hi/lo fp8 weights in the on-device layouts."""
    E = w1.shape[0]
    w13_e = np.empty((E, HC, P, 4, KC, P), dtype=F8)
    w2_e = np.empty((E, NGRP, P, NCW, NG), dtype=F8)

    def t13(a):  # [D, H] -> [HC, P(k), KC, P(h)]
        return a.reshape(KC, P, HC, P).transpose(2, 1, 0, 3)

    def t2(a):  # [H, D] -> [NGRP, P(h), HC, NG]
        return a.reshape(HC, P, NGRP, NG).transpose(2, 1, 0, 3)

    for e in range(E):
        h1, l1 = _hilo(w1[e] * SW1)
        h3, l3 = _hilo(w3[e] * SW3)
        w13_e[e, :, :, 0] = t13(h1)
        w13_e[e, :, :, 1] = t13(l1)
        w13_e[e, :, :, 2] = t13(h3)
        w13_e[e, :, :, 3] = t13(l3)
        h2, l2 = _hilo(w2[e] * SW2)
        th, tl = t2(h2), t2(l2)
        w2_e[e, :, :, 0:HC] = th          # wh0..wh10
        w2_e[e, :, :, HC] = tl[:, :, HC - 1]   # wl10
        w2_e[e, :, :, HC + 1:NCW] = tl[:, :, 0:HC - 1]  # wl0..wl9
    return w13_e, w2_e


def kernel(x, w1, w2, w3, m_sizes, _trace=False):
    x = np.asarray(x, dtype=np.float32)
    w1 = np.asarray(w1, dtype=np.float32)
    w2 = np.asarray(w2, dtype=np.float32)
    w3 = np.asarray(w3, dtype=np.float32)
    T = x.shape[0]
    assert x.shape[1] == D and w1.shape[1:] == (D, H), (x.shape, w1.shape)
    assert w2.shape[1:] == (H, D) and w3.shape[1:] == (D, H), (w2.shape, w3.shape)

    per_core, S = _plan(m_sizes, T)

    if S not in _compiled_cache:
        _compiled_cache[S] = _build_program(S)
    nc = _compiled_cache[S]

    wkey = (id(w1), id(w2), id(w3))
    if wkey not in _wq_cache:
        _wq_cache.clear()
        _wq_cache[wkey] = _quant_weights(w1, w2, w3)
    w13_e, w2_e = _wq_cache[wkey]

    in_maps = []
    for c in range(N_CORES):
        slots = per_core[c]
        xt_c = np.zeros((S, 2, P, KC, TOK), dtype=F8)
        for s, (e, off, ln) in enumerate(slots):
            if ln:
                seg = np.zeros((TOK, D), dtype=np.float32)
                seg[:ln] = x[off:off + ln]
                sh, sl = _hilo(seg)
                # [TOK, D] -> [P(k), KC, TOK]
                xt_c[s, 0] = sh.reshape(TOK, KC, P).transpose(2, 1, 0)
                xt_c[s, 1] = sl.reshape(TOK, KC, P).transpose(2, 1, 0)
        eids = [e for (e, _, _) in slots]
        in_maps.append({
            "xt": xt_c,
            "w13": np.ascontiguousarray(w13_e[eids]),
            "w2c": np.ascontiguousarray(w2_e[eids]),
        })

    try:
        res = run_bass_kernel_spmd(
            nc, in_maps, list(range(N_CORES)), trace=_trace,
        )
    except Exception:
        # transient NRT device errors have been observed once after a fresh
        # compile; a single retry is free if the device truly died
        res = run_bass_kernel_spmd(
            nc, in_maps, list(range(N_CORES)), trace=_trace,
        )

    full = np.zeros((T, D), dtype=np.float32)
    for c in range(N_CORES):
        oc = res.results[c]["out"].astype(np.float32)
        for s, (e, off, ln) in enumerate(per_core[c]):
            if ln:
                full[off:off + ln] = oc[s * TOK:s * TOK + ln]

    last_run_info.clear()
    last_run_info.update({
        "exec_time_ns": res.exec_time_ns,
        "profile_json": getattr(res, "profile_json", None),
        "S": S,
    })
    return full
